# revision 33
# baseline (speedup 1.0000x reference)
"""Ernie4 decoder layer (RMSNorm + GQA attention + shared expert + 16-expert
top-2 MoE) on 8 Trainium2 NeuronCores.

v2 design:
  - Attention head-parallel (2 q-heads + 1 kv-head per core), fp16 matmuls,
    causal block-skipping; o_proj partials combined with an fp16 ReduceScatter.
  - Router runs per-core on the fp32 post-attention x (own token block) BEFORE
    the AllGather; normalized top-2 weights are packed into the AllGather
    payload ([x fp16 | w fp16]) so selections are bit-identical across cores.
  - One AllGather of the packed payload; x^T built on-device (PE transposes)
    and kept resident in SBUF for router prefix-sums, shared expert and MoE.
  - Shared expert intermediate-sharded (256/core); its output initializes the
    combine buffer. Experts (2/core) gather tokens by indirect DMA, compute
    gate/up/down in fp16 with 1024-wide moving operands, scatter-add back.
  - Second fp16 ReduceScatter produces the final token-sharded output.
"""
import sys
sys.path.insert(0, "/opt/trn_rl_repo")

import numpy as np

import concourse.bass as bass
import concourse.bacc as bacc
import concourse.tile as tile
import concourse.mybir as mybir
from concourse import bass_utils

dt = mybir.dt
F32 = dt.float32
F32R = dt.float32r
F16 = dt.float16
I32 = dt.int32
AF = mybir.ActivationFunctionType
ALU = mybir.AluOpType
AX = mybir.AxisListType

T, H, NH, NKV, D = 1024, 2048, 16, 4, 128
E, I, IS = 16, 1024, 2048
ISC = IS // 8           # shared-expert intermediate per core
EPS = 1e-6
THETA = 10000.0
NCN = 8
P = 128
TB = T // P             # 8 token blocks
HC = H // P             # 16 hidden chunks
IP = I // P             # 8 expert-intermediate chunks
CAP = 256               # per-expert token capacity
PW = H + E              # AllGather payload width (x | w_topk)
BIG = 1.0e6
NEG = -30000.0          # fp16-safe mask value
RG = [list(range(NCN))]


def _emit(nc, tc):
    ex = {}
    for name, shape, d in [
        ("hid", [T, H], F32), ("hid_slice", [P, H], F32),
        ("w_qkv_pk", [P, HC * 512], F16),
        ("wo0", [D, H], F16), ("wo1", [D, H], F16),
        ("cosq", [D, T], F16), ("sinq", [D, T], F16),
        ("cosk", [D, T], F16), ("sink", [D, T], F16),
        ("permh", [P, P], F16), ("identh_in", [P, P], F16),
        ("identr_in", [P, P], F32), ("diagmask", [P, P], F16),
        ("gate_w_pk", [P, HC * E], F32), ("gate_b", [P, E], F32),
        ("emask0", [P, E], F32), ("emask1", [P, E], F32),
        ("ut_in", [P, P], F16), ("slb_in", [8, TB * P], F16),
        ("ws_g_pk", [P, HC * ISC], F16), ("ws_u_pk", [P, HC * ISC], F16),
        ("ws_d", [ISC, H], F16),
        ("we_g", [2, H, I], F16), ("we_u", [2, H, I], F16),
        ("we_d", [2, I, H], F16),
    ]:
        ex[name] = nc.dram_tensor(name, shape, d, kind="ExternalInput").ap()
    out_slice = nc.dram_tensor("out_slice", [P, H], F32, kind="ExternalOutput").ap()
    res_slice = nc.dram_tensor("res_slice", [P, H], F32, kind="ExternalOutput").ap()

    with tc.tile_pool(name="pp", bufs=1) as pp, \
         tc.tile_pool(name="dram", bufs=1, space="DRAM") as dram:
        rs_in = dram.tile([T, H], F16)
        rs_out = dram.tile([P, H], F16)
        ag_in = dram.tile([P, PW], F16)
        x_tm = dram.tile([T, PW], F16, addr_space="Shared")
        tok_lists = dram.tile([2 * CAP, 1], I32)
        rs2h = [dram.tile([T, 1024], F16, name=f"rs2h{i}") for i in range(2)]
        rs2ho = [dram.tile([P, 1024], F16, name=f"rs2ho{i}") for i in range(2)]

        identh = pp.tile([P, P], F16)
        nc.sync.dma_start(identh[:], ex["identh_in"][:])
        identf = pp.tile([P, P], F32)
        nc.sync.dma_start(identf[:], ex["identr_in"][:])
        eps_t = pp.tile([P, 1], F32)
        nc.vector.memset(eps_t[:], EPS)

        # ======== Phase A: norm + transpose + QKV + rope ========
        with tc.tile_pool(name="pab", bufs=1) as pab:
            qT = [pab.tile([P, T], F16, tag=f"qT{j}", name=f"qT{j}")
                  for j in range(2)]
            kT = pab.tile([P, T], F16)
            v_tm = pab.tile([P, TB * D], F16)
            wo_sb = [pab.tile([P, H], F16, tag=f"wo{j}", name=f"wo{j}")
                     for j in range(2)]
            diagm = pab.tile([P, P], F16)

            with tc.tile_pool(name="pa", bufs=1) as pa, \
                 tc.tile_pool(name="pa2", bufs=2) as pa2:
                # hid prefetch first so norm can start immediately
                hidbs = []
                for b in range(TB):
                    t_ = pa2.tile([P, H], F32, tag="hidb", bufs=8,
                                  name=f"hidb{b}")
                    nc.sync.dma_start(t_[:], ex["hid"][b * P:(b + 1) * P, :])
                    hidbs.append(t_)
                nc.sync.dma_start(wo_sb[0][:], ex["wo0"][:])
                nc.sync.dma_start(wo_sb[1][:], ex["wo1"][:])
                nc.sync.dma_start(diagm[:], ex["diagmask"][:])
                cosq = pa.tile([D, T], F16)
                sinq = pa.tile([D, T], F16)
                cosk = pa.tile([D, T], F16)
                sink = pa.tile([D, T], F16)
                for t_, s_ in [(cosq, "cosq"), (sinq, "sinq"),
                               (cosk, "cosk"), (sink, "sink")]:
                    nc.sync.dma_start(t_[:], ex[s_][:])
                permh = pa.tile([P, P], F16)
                nc.sync.dma_start(permh[:], ex["permh"][:])
                wqkv_sb = pa.tile([P, HC * 512], F16)
                nc.sync.dma_start(wqkv_sb[:], ex["w_qkv_pk"][:])

                x0T = [pa.tile([P, T], F16, tag=f"x0T{hc}", name=f"x0T{hc}")
                       for hc in range(HC)]
                qraw = [pa.tile([P, T], F16, tag=f"qraw{j}", name=f"qraw{j}")
                        for j in range(2)]
                kraw = pa.tile([P, T], F16)
                vraw = pa.tile([P, T], F16)
                dump = pa.tile([P, H], F32)

                with tc.tile_pool(name="psA1", bufs=2, space="PSUM") as psA1, \
                     tc.tile_pool(name="psA2", bufs=1, space="PSUM") as psA2:
                    for n in range(2):
                        x0hs = []
                        for bb in range(TB // 2):
                            b = n * (TB // 2) + bb
                            hidb = hidbs[b]
                            ssum = pa2.tile([P, 1], F32, tag="ssum")
                            nc.scalar.activation(dump[:], hidb[:], AF.Square,
                                                 accum_out=ssum[:, :1])
                            rms = pa2.tile([P, 1], F32, tag="rms")
                            nc.scalar.activation(rms[:], ssum[:], AF.Sqrt,
                                                 bias=eps_t[:, :1],
                                                 scale=1.0 / H)
                            inv = pa2.tile([P, 1], F32, tag="inv")
                            nc.vector.reciprocal(inv[:], rms[:])
                            x0h = pa2.tile([P, H], F16, tag="x0h", bufs=5,
                                           name=f"x0h{b}")
                            nc.vector.tensor_scalar_mul(x0h[:], hidb[:],
                                                        inv[:, :1])
                            x0hs.append(x0h)
                        sl = slice(n * 512, (n + 1) * 512)
                        for hc in range(HC):
                            tp = psA1.tile([P, 512], F16, tag="tpA")
                            for bb in range(4):
                                nc.tensor.transpose(
                                    tp[:, bb * P:(bb + 1) * P],
                                    x0hs[bb][:, hc * P:(hc + 1) * P],
                                    identh[:])
                            if hc % 2 == 0:
                                nc.vector.tensor_copy(x0T[hc][:, sl], tp[:])
                            else:
                                nc.scalar.activation(x0T[hc][:, sl], tp[:],
                                                     AF.Copy)
                        # QKV for this half of T
                        for j, (c0, dst) in enumerate(
                                [(0, qraw[0]), (128, qraw[1]),
                                 (256, kraw), (384, vraw)]):
                            ps = psA2.tile([P, 512], F32, tag=f"qkv{j}",
                                           name=f"qkv{j}")
                            for hc in range(HC):
                                nc.tensor.matmul(
                                    ps[:],
                                    wqkv_sb[:, hc * 512 + c0:hc * 512 + c0 + P],
                                    x0T[hc][:, sl],
                                    start=(hc == 0), stop=(hc == HC - 1))
                            nc.vector.tensor_copy(dst[:, sl], ps[:])

                # rope + v transpose
                with tc.tile_pool(name="psA3", bufs=2, space="PSUM") as psA3, \
                     tc.tile_pool(name="psA4", bufs=2, space="PSUM") as psA4:
                    for src, dst, c_, s_ in [(qraw[0], qT[0], cosq, sinq),
                                             (qraw[1], qT[1], cosq, sinq),
                                             (kraw, kT, cosk, sink)]:
                        sw = psA3.tile([P, T], F32, tag="sw")
                        for nn in range(2):
                            sl = slice(nn * 512, (nn + 1) * 512)
                            nc.tensor.matmul(sw[:, sl], permh[:], src[:, sl],
                                             start=True, stop=True)
                        t1 = pa2.tile([P, T], F16, tag="ropet1")
                        nc.vector.tensor_mul(t1[:], src[:], c_[:])
                        t2 = pa2.tile([P, T], F16, tag="ropet2")
                        nc.vector.tensor_mul(t2[:], sw[:], s_[:])
                        nc.vector.tensor_add(dst[:], t1[:], t2[:])
                    for g4 in range(2):
                        tp = psA4.tile([P, 512], F16, tag="tpV")
                        for bb in range(4):
                            b = g4 * 4 + bb
                            nc.tensor.transpose(
                                tp[:, bb * P:(bb + 1) * P],
                                vraw[:, b * P:(b + 1) * P], identh[:])
                        nc.vector.tensor_copy(
                            v_tm[:, g4 * 512:(g4 + 1) * 512], tp[:])

            # ======== Phase B: attention + o_proj ========
            with tc.tile_pool(name="pb", bufs=1) as pb, \
                 tc.tile_pool(name="pb2", bufs=2) as pb2:
                attnT = pb.tile([P, TB * T], F16)
                attnT3 = attnT[:].rearrange("p (kc t) -> p kc t", kc=TB)
                oT = [pb.tile([P, T], F16, tag=f"oT{j}", name=f"oT{j}")
                      for j in range(2)]
                with tc.tile_pool(name="psBs", bufs=2, space="PSUM") as psBs, \
                     tc.tile_pool(name="psBt", bufs=2, space="PSUM") as psBt, \
                     tc.tile_pool(name="psBo", bufs=1, space="PSUM") as psBo:
                    for h in range(2):
                        prev = None
                        for qc in range(TB + 1):
                            cur = None
                            if qc < TB:
                                W = (qc + 1) * P
                                sc = psBs.tile([P, T], F32, tag="sc")
                                for c0 in range(0, W, 512):
                                    c1 = min(c0 + 512, W)
                                    nc.tensor.matmul(
                                        sc[:, c0:c1],
                                        qT[h][:, qc * P:(qc + 1) * P],
                                        kT[:, c0:c1], start=True, stop=True)
                                nc.vector.tensor_tensor(
                                    out=sc[:, W - P:W], in0=sc[:, W - P:W],
                                    in1=diagm[:], op=ALU.add)
                                probs = pb2.tile([P, T], F16, tag="probs")
                                ssum = pb2.tile([P, 1], F32, tag="esum")
                                nc.scalar.activation(probs[:, :W], sc[:, :W],
                                                     AF.Exp,
                                                     accum_out=ssum[:, :1])
                                rec = pb2.tile([P, 1], F32, tag="rec")
                                nc.vector.reciprocal(rec[:], ssum[:])
                                nc.vector.tensor_scalar_mul(probs[:, :W],
                                                            probs[:, :W],
                                                            rec[:, :1])
                                cur = (probs, qc)
                            if prev is not None:
                                pprobs, pqc = prev
                                for g4 in range(0, pqc + 1, 4):
                                    cnt = min(4, pqc + 1 - g4)
                                    tp = psBt.tile([P, 512], F16, tag="tpB")
                                    for i in range(cnt):
                                        kc = g4 + i
                                        nc.tensor.transpose(
                                            tp[:, i * P:(i + 1) * P],
                                            pprobs[:, kc * P:(kc + 1) * P],
                                            identh[:])
                                    dst = attnT3[:, g4:g4 + cnt,
                                                 pqc * P:(pqc + 1) * P]
                                    src = tp[:, :cnt * P].rearrange(
                                        "p (kc t) -> p kc t", kc=cnt)
                                    if pqc % 2 == 0:
                                        nc.vector.tensor_copy(dst, src)
                                    else:
                                        nc.scalar.activation(dst, src,
                                                             AF.Copy)
                            prev = cur
                        ov = psBo.tile([P, T], F32, tag="ov")
                        for nn in range(2):
                            lo, hi = nn * 512, (nn + 1) * 512
                            kcs = [kc for kc in range(TB) if kc * P < hi]
                            for kc in kcs:
                                c0 = max(kc * P, lo)
                                nc.tensor.matmul(
                                    ov[:, c0:hi],
                                    v_tm[:, kc * P:(kc + 1) * P],
                                    attnT[:, kc * T + c0:kc * T + hi],
                                    start=(kc == 0),
                                    stop=(kc == kcs[-1]),
                                    skip_group_check=True)
                        nc.vector.tensor_copy(oT[h][:], ov[:])
                # o_proj
                with tc.tile_pool(name="psBp", bufs=2, space="PSUM") as psBp:
                    for tb_ in range(TB):
                        for nn in range(2):
                            sl = slice(nn * 1024, (nn + 1) * 1024)
                            ps = psBp.tile([P, 1024], F32, tag="psO")
                            for h in range(2):
                                for q2 in range(2):
                                    s2 = slice(q2 * 512, (q2 + 1) * 512)
                                    nc.tensor.matmul(
                                        ps[:, s2],
                                        oT[h][:, tb_ * P:(tb_ + 1) * P],
                                        wo_sb[h][:, nn * 1024 + q2 * 512:
                                                  nn * 1024 + (q2 + 1) * 512],
                                        start=(h == 0), stop=(h == 1))
                            ob = pb2.tile([P, 1024], F16, tag="ob")
                            nc.vector.tensor_copy(ob[:], ps[:])
                            nc.sync.dma_start(
                                rs_in[tb_ * P:(tb_ + 1) * P, sl], ob[:])

        # ======== weight prefetch pool (covers rest of kernel) ========
        with tc.tile_pool(name="pw", bufs=1) as pw:
            xT = [pw.tile([P, T], F16, tag=f"xT{hc}", name=f"xT{hc}")
                  for hc in range(HC)]
            wd_res = [pw.tile([P, H], F16, tag=f"wd{e}_{ip}",
                              name=f"wd{e}_{ip}")
                      for e in range(2) for ip in range(IP)]
            for e in range(2):
                for ip in range(IP):
                    nc.sync.dma_start(
                        wd_res[e * IP + ip][:],
                        ex["we_d"][e, ip * P:(ip + 1) * P, :])
            wsg_sb = pw.tile([P, HC * ISC], F16)
            nc.sync.dma_start(wsg_sb[:], ex["ws_g_pk"][:])
            wsu_sb = pw.tile([P, HC * ISC], F16)
            nc.sync.dma_start(wsu_sb[:], ex["ws_u_pk"][:])
            wsd_sb = [pw.tile([P, H], F16, tag=f"wsd{sp}", name=f"wsd{sp}")
                      for sp in range(2)]
            for sp in range(2):
                nc.sync.dma_start(wsd_sb[sp][:],
                                  ex["ws_d"][sp * P:(sp + 1) * P, :])
            gwr = pw.tile([P, HC * E], F32)
            nc.sync.dma_start(gwr[:], ex["gate_w_pk"][:])
            gate_b_sb = pw.tile([P, E], F32)
            nc.sync.dma_start(gate_b_sb[:], ex["gate_b"][:])
            ut_sb = pw.tile([P, P], F16)
            nc.sync.dma_start(ut_sb[:], ex["ut_in"][:])
            slb_sb = pw.tile([8, TB * P], F16)
            nc.sync.dma_start(slb_sb[:], ex["slb_in"][:])
            em = [pw.tile([P, E], F32, tag=f"em{e}", name=f"em{e}")
                  for e in range(2)]
            nc.sync.dma_start(em[0][:], ex["emask0"][:])
            nc.sync.dma_start(em[1][:], ex["emask1"][:])

            nc.gpsimd.collective_compute(
                "ReduceScatter", ALU.add, ins=[rs_in.opt()],
                outs=[rs_out.opt()], replica_groups=RG)

            # ======== Phase D: residual + norm + local router ========
            with tc.tile_pool(name="pd", bufs=1) as pd:
                hid_sl = pd.tile([P, H], F32)
                nc.sync.dma_start(hid_sl[:], ex["hid_slice"][:])
                attn_sl = pd.tile([P, H], F16)
                nc.sync.dma_start(attn_sl[:], rs_out[:])
                res_sb = pd.tile([P, H], F32)
                nc.vector.tensor_add(res_sb[:], hid_sl[:], attn_sl[:])
                nc.sync.dma_start(res_slice[:], res_sb[:])
                dump2 = pd.tile([P, H], F32)
                ssum = pd.tile([P, 1], F32)
                nc.scalar.activation(dump2[:], res_sb[:], AF.Square,
                                     accum_out=ssum[:, :1])
                rms = pd.tile([P, 1], F32)
                nc.scalar.activation(rms[:], ssum[:], AF.Sqrt,
                                     bias=eps_t[:, :1], scale=1.0 / H)
                inv = pd.tile([P, 1], F32)
                nc.vector.reciprocal(inv[:], rms[:])
                x_sl = pd.tile([P, H], F32)
                nc.vector.tensor_scalar_mul(x_sl[:], res_sb[:], inv[:, :1])
                pay = pd.tile([P, PW], F16)
                nc.vector.tensor_copy(pay[:, :H], x_sl[:])
                # local router on fp32 x
                xsT = pd.tile([P, HC * P], F32)
                with tc.tile_pool(name="psDt", bufs=2, space="PSUM") as psDt:
                    for g4 in range(4):
                        tp = psDt.tile([P, 512], F32, tag="tpD")
                        for i in range(4):
                            hc = g4 * 4 + i
                            nc.tensor.transpose(
                                tp[:, i * P:(i + 1) * P],
                                x_sl[:, hc * P:(hc + 1) * P], identf[:])
                        nc.vector.tensor_copy(
                            xsT[:, g4 * 512:(g4 + 1) * 512], tp[:])
                with tc.tile_pool(name="psDr", bufs=1, space="PSUM") as psDr:
                    lg = psDr.tile([P, E], F32, tag="lg")
                    for hc in range(HC):
                        nc.tensor.matmul(lg[:], xsT[:, hc * P:(hc + 1) * P],
                                         gwr[:, hc * E:(hc + 1) * E],
                                         start=(hc == 0), stop=(hc == HC - 1))
                    sig = pd.tile([P, E], F32)
                    nc.scalar.activation(sig[:], lg[:], AF.Sigmoid)
                sb_ = pd.tile([P, E], F32)
                nc.vector.tensor_add(sb_[:], sig[:], gate_b_sb[:])
                mx8 = pd.tile([P, 8], F32)
                nc.vector.max(out=mx8[:], in_=sb_[:])
                s1 = pd.tile([P, E], F32)
                nc.vector.tensor_tensor(out=s1[:], in0=sb_[:],
                                        in1=mx8[:, 0:1].to_broadcast([P, E]),
                                        op=ALU.is_equal)
                s2 = pd.tile([P, E], F32)
                nc.vector.tensor_tensor(out=s2[:], in0=sb_[:],
                                        in1=mx8[:, 1:2].to_broadcast([P, E]),
                                        op=ALU.is_equal)
                nc.vector.tensor_add(s1[:], s1[:], s2[:])
                nc.vector.tensor_scalar_min(s1[:], s1[:], 1.0)
                wa = pd.tile([P, E], F32)
                nc.vector.tensor_mul(wa[:], s1[:], sig[:])
                nrm = pd.tile([P, 1], F32)
                nc.vector.reduce_sum(nrm[:], wa[:], axis=AX.X)
                rec = pd.tile([P, 1], F32)
                nc.vector.reciprocal(rec[:], nrm[:])
                w_tm = pd.tile([P, E], F32)
                nc.vector.tensor_scalar_mul(w_tm[:], wa[:], rec[:, :1])
                nc.vector.tensor_copy(pay[:, H:], w_tm[:])
                nc.sync.dma_start(ag_in[:], pay[:])

            nc.gpsimd.collective_compute(
                "AllGather", ALU.bypass, ins=[ag_in.opt()],
                outs=[x_tm.opt()], replica_groups=RG)

            # ======== Phase X: build x^T in SBUF + global router ========
            with tc.tile_pool(name="px", bufs=1) as px, \
                 tc.tile_pool(name="px2", bufs=2) as px2:
                totals = px.tile([8, E], F16)
                grank = [px.tile([P, E], F32, tag=f"grank{b}",
                                 name=f"grank{b}") for b in range(TB)]
                selb = [px.tile([P, E], F16, tag=f"selb{b}",
                                name=f"selb{b}") for b in range(TB)]
                pre_sb = [px.tile([P, E], F16, tag=f"pre{b}",
                                  name=f"pre{b}") for b in range(TB)]
                with tc.tile_pool(name="psXt", bufs=2, space="PSUM") as psXt, \
                     tc.tile_pool(name="psXr", bufs=2, space="PSUM") as psXr:
                    xbs = []
                    for b in range(TB):
                        xb = px2.tile([P, PW], F16, tag="xb", bufs=8,
                                      name=f"xb{b}")
                        nc.sync.dma_start(xb[:],
                                          x_tm[b * P:(b + 1) * P, :])
                        xbs.append(xb)
                    for b in range(TB):
                        xb = xbs[b]
                        nc.vector.tensor_scalar(out=selb[b][:],
                                                in0=xb[:, H:PW],
                                                scalar1=0.0, scalar2=1.0,
                                                op0=ALU.is_gt, op1=ALU.mult)
                        pr = psXr.tile([P, E], F32, tag="pr")
                        nc.tensor.matmul(pr[:], ut_sb[:], selb[b][:],
                                         start=True, stop=True)
                        nc.vector.tensor_copy(pre_sb[b][:], pr[:])
                        nc.sync.dma_start(totals[b:b + 1, :],
                                          pre_sb[b][127:128, :])
                    for n in range(2):
                        sl = slice(n * 512, (n + 1) * 512)
                        for hc in range(HC):
                            tp = psXt.tile([P, 512], F16, tag="tpX")
                            for bb in range(4):
                                b = n * 4 + bb
                                nc.tensor.transpose(
                                    tp[:, bb * P:(bb + 1) * P],
                                    xbs[b][:, hc * P:(hc + 1) * P],
                                    identh[:])
                            if hc % 2 == 0:
                                nc.vector.tensor_copy(xT[hc][:, sl], tp[:])
                            else:
                                nc.scalar.activation(xT[hc][:, sl], tp[:],
                                                     AF.Copy)
                    for b in range(TB):
                        ofs = psXr.tile([P, E], F32, tag="ofs")
                        nc.tensor.matmul(ofs[:], slb_sb[:, b * P:(b + 1) * P],
                                         totals[:], start=True, stop=True)
                        nc.vector.tensor_tensor(out=grank[b][:],
                                                in0=pre_sb[b][:], in1=ofs[:],
                                                op=ALU.add)
                        gm = px2.tile([P, E], F32, tag="gm")
                        nc.vector.tensor_scalar(out=gm[:], in0=grank[b][:],
                                                scalar1=float(CAP),
                                                scalar2=BIG,
                                                op0=ALU.is_gt, op1=ALU.mult)
                        nc.vector.tensor_add(grank[b][:], grank[b][:], gm[:])
                        um = px2.tile([P, E], F32, tag="um")
                        nc.vector.tensor_scalar(out=um[:], in0=selb[b][:],
                                                scalar1=-BIG, scalar2=BIG,
                                                op0=ALU.mult, op1=ALU.add)
                        nc.vector.tensor_add(grank[b][:], grank[b][:], um[:])
                # token lists (grank is 1-based; -1 folded into scatter base)
                sent = px.tile([P, 1], I32)
                nc.vector.memset(sent[:], 1000000)
                for kk in range(2 * CAP // P):
                    nc.sync.dma_start(tok_lists[kk * P:(kk + 1) * P, :],
                                      sent[:])
                for b in range(TB):
                    tok = px2.tile([P, 1], I32, tag="tok")
                    nc.gpsimd.iota(tok[:], pattern=[[0, 1]], base=b * P,
                                   channel_multiplier=1)
                    for ei in range(2):
                        gsel = px2.tile([P, E], F32, tag="gsel")
                        nc.vector.tensor_mul(gsel[:], grank[b][:], em[ei][:])
                        ridx = px2.tile([P, 1], F32, tag="ridx")
                        nc.vector.reduce_sum(ridx[:], gsel[:], axis=AX.X)
                        nc.vector.tensor_scalar_add(ridx[:], ridx[:],
                                                    float(ei * CAP - 1))
                        ridx_i = px2.tile([P, 1], I32, tag="ridxi")
                        nc.vector.tensor_copy(ridx_i[:], ridx[:])
                        nc.gpsimd.indirect_dma_start(
                            out=tok_lists[:],
                            out_offset=bass.IndirectOffsetOnAxis(
                                ap=ridx_i[:, :1], axis=0),
                            in_=tok[:], in_offset=None,
                            bounds_check=2 * CAP - 1, oob_is_err=False)

            # ======== Expert gathers (both experts, before shared) ========
            with tc.tile_pool(name="pg", bufs=1) as pg, \
                 tc.tile_pool(name="pg2", bufs=2) as pg2:
                idx_sb = [[pg.tile([P, 1], I32, tag=f"idx{ei}_{k}",
                                   name=f"idx{ei}_{k}") for k in range(2)]
                          for ei in range(2)]
                gxT = [pg.tile([P, HC * 2 * P], F16, tag=f"gxT{ei}",
                               name=f"gxT{ei}") for ei in range(2)]
                wcol = [[pg.tile([P, 1], F32, tag=f"wcol{ei}_{k}",
                                 name=f"wcol{ei}_{k}") for k in range(2)]
                        for ei in range(2)]
                with tc.tile_pool(name="psFt", bufs=2, space="PSUM") as psFt:
                    for ei in range(2):
                        for k in range(2):
                            nc.sync.dma_start(
                                idx_sb[ei][k][:],
                                tok_lists[ei * CAP + k * P:
                                          ei * CAP + (k + 1) * P, :])
                            gx = pg2.tile([P, PW], F16, tag="gx")
                            nc.vector.memset(gx[:], 0.0)
                            nc.gpsimd.indirect_dma_start(
                                out=gx[:], out_offset=None,
                                in_=x_tm[:],
                                in_offset=bass.IndirectOffsetOnAxis(
                                    ap=idx_sb[ei][k][:, :1], axis=0),
                                bounds_check=T - 1, oob_is_err=False)
                            wtmp = pg2.tile([P, E], F32, tag="wtmp")
                            nc.vector.tensor_mul(wtmp[:], gx[:, H:PW],
                                                 em[ei][:])
                            nc.vector.reduce_sum(wcol[ei][k][:], wtmp[:],
                                                 axis=AX.X)
                            for g4 in range(4):
                                tp = psFt.tile([P, 512], F16, tag="tpF")
                                for i in range(4):
                                    hc = g4 * 4 + i
                                    nc.tensor.transpose(
                                        tp[:, i * P:(i + 1) * P],
                                        gx[:, hc * P:(hc + 1) * P],
                                        identh[:])
                                dst = gxT[ei][:, k * HC * P + g4 * 512:
                                              k * HC * P + (g4 + 1) * 512]
                                if g4 % 2 == 0:
                                    nc.vector.tensor_copy(dst, tp[:])
                                else:
                                    nc.scalar.activation(dst, tp[:], AF.Copy)

                # ======== Shared expert ========
                hs = [pg.tile([P, T], F16, tag=f"hs{sp}", name=f"hs{sp}")
                      for sp in range(2)]
                with tc.tile_pool(name="psS", bufs=1, space="PSUM") as psS:
                    gps = [psS.tile([P, T], F32, tag=f"sgp{sp}",
                                    name=f"sgp{sp}") for sp in range(2)]
                    ups = [psS.tile([P, T], F32, tag=f"sup{sp}",
                                    name=f"sup{sp}") for sp in range(2)]
                    for hc in range(HC):
                        for sp in range(2):
                            c0 = hc * ISC + sp * P
                            for nn in range(2):
                                sl = slice(nn * 512, (nn + 1) * 512)
                                nc.tensor.matmul(gps[sp][:, sl],
                                                 wsg_sb[:, c0:c0 + P],
                                                 xT[hc][:, sl],
                                                 start=(hc == 0),
                                                 stop=(hc == HC - 1))
                                nc.tensor.matmul(ups[sp][:, sl],
                                                 wsu_sb[:, c0:c0 + P],
                                                 xT[hc][:, sl],
                                                 start=(hc == 0),
                                                 stop=(hc == HC - 1))
                    for sp in range(2):
                        ga = pg2.tile([P, T], F16, tag="ga")
                        nc.scalar.activation(ga[:], gps[sp][:], AF.Silu)
                        nc.vector.tensor_mul(hs[sp][:], ga[:], ups[sp][:])
                with tc.tile_pool(name="psS3", bufs=2, space="PSUM") as psS3:
                    for nn in range(2):
                        for tb_ in range(TB):
                            ps3 = psS3.tile([P, 1024], F32, tag="psSd")
                            for sp in range(2):
                                for q2 in range(2):
                                    s2 = slice(q2 * 512, (q2 + 1) * 512)
                                    nc.tensor.matmul(
                                        ps3[:, s2],
                                        hs[sp][:, tb_ * P:(tb_ + 1) * P],
                                        wsd_sb[sp][:, nn * 1024 + q2 * 512:
                                                    nn * 1024 + (q2 + 1) * 512],
                                        start=(sp == 0), stop=(sp == 1))
                            sd = pg2.tile([P, 1024], F16, tag="sd")
                            nc.vector.tensor_copy(sd[:], ps3[:])
                            nc.sync.dma_start(
                                rs2h[nn][tb_ * P:(tb_ + 1) * P, :], sd[:])

                # ======== Experts: gate/up for both, then down by half ======
                h_sbs = []
                for ei in range(2):
                    with tc.tile_pool(name=f"pfw{ei}", bufs=1) as pfw, \
                         tc.tile_pool(name=f"psF1{ei}", bufs=1,
                                      space="PSUM") as psF1:
                        h_tm = [pg.tile([P, I], F16, tag=f"htm{ei}_{k}",
                                        name=f"htm{ei}_{k}")
                                for k in range(2)]
                        g_ps = [psF1.tile([P, I], F32, tag=f"fg{k}",
                                          name=f"fg{k}_{ei}")
                                for k in range(2)]
                        u_ps = [psF1.tile([P, I], F32, tag=f"fu{k}",
                                          name=f"fu{k}_{ei}")
                                for k in range(2)]
                        for hc in range(HC):
                            wg = pfw.tile([P, I], F16, tag="wg", bufs=5)
                            nc.sync.dma_start(
                                wg[:], ex["we_g"][ei, hc * P:(hc + 1) * P, :])
                            wu = pfw.tile([P, I], F16, tag="wu", bufs=5)
                            nc.sync.dma_start(
                                wu[:], ex["we_u"][ei, hc * P:(hc + 1) * P, :])
                            for k in range(2):
                                c0 = k * HC * P + hc * P
                                for nn in range(2):
                                    sl = slice(nn * 512, (nn + 1) * 512)
                                    nc.tensor.matmul(g_ps[k][:, sl],
                                                     gxT[ei][:, c0:c0 + P],
                                                     wg[:, sl],
                                                     start=(hc == 0),
                                                     stop=(hc == HC - 1))
                                    nc.tensor.matmul(u_ps[k][:, sl],
                                                     gxT[ei][:, c0:c0 + P],
                                                     wu[:, sl],
                                                     start=(hc == 0),
                                                     stop=(hc == HC - 1))
                        for k in range(2):
                            sg = pg2.tile([P, I], F16, tag="sg")
                            nc.scalar.activation(sg[:], g_ps[k][:], AF.Silu)
                            nc.vector.tensor_mul(h_tm[k][:], sg[:],
                                                 u_ps[k][:])
                    h_sb = pg.tile([P, IP * 2 * P], F16, tag=f"hsb{ei}",
                                   name=f"hsb{ei}")
                    with tc.tile_pool(name=f"psF2{ei}", bufs=2,
                                      space="PSUM") as psF2:
                        for k in range(2):
                            for g4 in range(2):
                                tp = psF2.tile([P, 512], F16, tag="tpF2")
                                for i in range(4):
                                    ip = g4 * 4 + i
                                    nc.tensor.transpose(
                                        tp[:, i * P:(i + 1) * P],
                                        h_tm[k][:, ip * P:(ip + 1) * P],
                                        identh[:])
                                dst = h_sb[:, k * IP * P + g4 * 512:
                                           k * IP * P + (g4 + 1) * 512]
                                if g4 % 2 == 0:
                                    nc.vector.tensor_copy(dst, tp[:])
                                else:
                                    nc.scalar.activation(dst, tp[:],
                                                         AF.Copy)
                    h_sbs.append(h_sb)
                with tc.tile_pool(name="psF3", bufs=2, space="PSUM") as psF3:
                    for nn in range(2):
                        for ei in range(2):
                            for k in range(2):
                                psd = psF3.tile([P, 1024], F32, tag="fd")
                                for ip in range(IP):
                                    c0 = k * IP * P + ip * P
                                    for q2 in range(2):
                                        s2 = slice(q2 * 512, (q2 + 1) * 512)
                                        nc.tensor.matmul(
                                            psd[:, s2],
                                            h_sbs[ei][:, c0:c0 + P],
                                            wd_res[ei * IP + ip][
                                                :, nn * 1024 + q2 * 512:
                                                nn * 1024 + (q2 + 1) * 512],
                                            start=(ip == 0),
                                            stop=(ip == IP - 1))
                                oh = pg2.tile([P, 1024], F16, tag="oh")
                                nc.vector.tensor_scalar_mul(
                                    oh[:], psd[:], wcol[ei][k][:, :1])
                                nc.gpsimd.indirect_dma_start(
                                    out=rs2h[nn][:],
                                    out_offset=bass.IndirectOffsetOnAxis(
                                        ap=idx_sb[ei][k][:, :1], axis=0),
                                    in_=oh[:], in_offset=None,
                                    bounds_check=T - 1, oob_is_err=False,
                                    compute_op=ALU.add)
                        nc.gpsimd.collective_compute(
                            "ReduceScatter", ALU.add, ins=[rs2h[nn].opt()],
                            outs=[rs2ho[nn].opt()], replica_groups=RG)

            with tc.tile_pool(name="pz", bufs=1) as pz:
                fin32 = pz.tile([P, H], F32)
                for nn in range(2):
                    fin = pz.tile([P, 1024], F16, tag="fin", bufs=2,
                                  name=f"fin{nn}")
                    nc.sync.dma_start(fin[:], rs2ho[nn][:])
                    nc.vector.tensor_copy(
                        fin32[:, nn * 1024:(nn + 1) * 1024], fin[:])
                nc.sync.dma_start(out_slice[:], fin32[:])


_CACHE = {}


def _build():
    key = "nc"
    if key in _CACHE:
        return _CACHE[key]
    nc = bacc.Bacc("TRN2", target_bir_lowering=False, debug=False,
                   num_devices=NCN)
    with tile.TileContext(nc) as tc:
        _emit(nc, tc)
    nc.compile()
    _CACHE[key] = nc
    return nc


def _host_prep(inputs):
    f16 = np.float16
    pos = np.asarray(inputs["positions"]).astype(np.float64)
    hid = np.asarray(inputs["hidden_states"], np.float32)
    w_in = np.asarray(inputs["w_in_ln"], np.float32)
    w_post = np.asarray(inputs["w_post_ln"], np.float32)
    wq = np.asarray(inputs["wq"], np.float32) * w_in[:, None]
    wk = np.asarray(inputs["wk"], np.float32) * w_in[:, None]
    wv = np.asarray(inputs["wv"], np.float32) * w_in[:, None]
    wo = np.asarray(inputs["wo"], np.float32)
    gate_w = np.asarray(inputs["gate_w"], np.float32) * w_post[None, :]
    gate_b = np.asarray(inputs["gate_bias"], np.float32).reshape(1, E)
    we_g = (np.asarray(inputs["we_gate"], np.float32)
            * w_post[None, :, None]).astype(f16)
    we_u = (np.asarray(inputs["we_up"], np.float32)
            * w_post[None, :, None]).astype(f16)
    we_d = np.asarray(inputs["we_down"], np.float32).astype(f16)
    ws_g = np.asarray(inputs["ws_gate"], np.float32) * w_post[:, None]
    ws_u = np.asarray(inputs["ws_up"], np.float32) * w_post[:, None]
    ws_d = np.asarray(inputs["ws_down"], np.float32).astype(f16)

    inv_freq = 1.0 / (THETA ** (np.arange(0, D, 2, dtype=np.float64) / D))
    f = pos[None, :] * inv_freq[:, None]
    cos2, sin2 = np.cos(f), np.sin(f)
    cosT = np.repeat(cos2, 2, axis=0).astype(np.float32)
    sinT = np.empty((D, T), np.float32)
    sinT[0::2] = -sin2
    sinT[1::2] = sin2
    s = 1.0 / np.sqrt(D)
    cosq, sinq = (cosT * s).astype(f16), (sinT * s).astype(f16)
    cosk, sink = cosT.astype(f16), sinT.astype(f16)

    ii = np.arange(P)
    diagmask = np.where(ii[:, None] >= ii[None, :], 0.0, NEG).astype(f16)
    ident = np.eye(P, dtype=np.float32)
    ut_in = np.triu(np.ones((P, P), np.float32)).astype(f16)
    slb_in = np.zeros((8, TB * P), np.float32)
    for b in range(TB):
        slb_in[:b, b * P:(b + 1) * P] = 1.0
    slb_in = slb_in.astype(f16)
    perm = np.zeros((P, P), np.float32)
    for i in range(0, P, 2):
        perm[i, i + 1] = 1.0
        perm[i + 1, i] = 1.0

    # packed (chunk-major) stationary layouts: [128, HC*width]
    def pack_pk(w, width):  # w: [H, width]
        return np.ascontiguousarray(
            w.reshape(HC, P, width).transpose(1, 0, 2).reshape(P, HC * width))

    gate_w_pk = pack_pk(gate_w.T.astype(np.float32), E)

    maps = []
    for c in range(NCN):
        g = c // 2
        w_qkv = pack_pk(np.concatenate([
            wq[:, 2 * c * D:(2 * c + 1) * D],
            wq[:, (2 * c + 1) * D:(2 * c + 2) * D],
            wk[:, g * D:(g + 1) * D],
            wv[:, g * D:(g + 1) * D]], axis=1), 512).astype(f16)
        em0 = np.zeros((P, E), np.float32)
        em0[:, 2 * c] = 1.0
        em1 = np.zeros((P, E), np.float32)
        em1[:, 2 * c + 1] = 1.0
        maps.append({
            "hid": hid,
            "hid_slice": np.ascontiguousarray(hid[c * P:(c + 1) * P]),
            "w_qkv_pk": w_qkv,
            "wo0": np.ascontiguousarray(wo[2 * c * D:(2 * c + 1) * D]).astype(f16),
            "wo1": np.ascontiguousarray(
                wo[(2 * c + 1) * D:(2 * c + 2) * D]).astype(f16),
            "cosq": cosq, "sinq": sinq, "cosk": cosk, "sink": sink,
            "permh": perm.astype(f16), "identh_in": ident.astype(f16),
            "identr_in": ident, "diagmask": diagmask,
            "gate_w_pk": gate_w_pk,
            "gate_b": np.broadcast_to(gate_b, (P, E)).astype(np.float32).copy(),
            "emask0": em0, "emask1": em1,
            "ut_in": ut_in, "slb_in": slb_in,
            "ws_g_pk": pack_pk(
                ws_g[:, c * ISC:(c + 1) * ISC].astype(np.float32), ISC
            ).astype(f16),
            "ws_u_pk": pack_pk(
                ws_u[:, c * ISC:(c + 1) * ISC].astype(np.float32), ISC
            ).astype(f16),
            "ws_d": np.ascontiguousarray(ws_d[c * ISC:(c + 1) * ISC]),
            "we_g": np.ascontiguousarray(we_g[2 * c:2 * c + 2]),
            "we_u": np.ascontiguousarray(we_u[2 * c:2 * c + 2]),
            "we_d": np.ascontiguousarray(we_d[2 * c:2 * c + 2]),
        })
    return maps


def kernel(trace=False, **inputs):
    nc = _build()
    maps = _host_prep(inputs)
    res = bass_utils.run_bass_kernel_spmd(
        nc, maps, core_ids=list(range(NCN)), trace=trace)
    out = np.concatenate([res.results[c]["out_slice"] for c in range(NCN)], 0)
    resid = np.concatenate([res.results[c]["res_slice"] for c in range(NCN)], 0)
    kernel.last_results = res
    return out, resid


# revision 38
# speedup vs baseline: 1.0695x; 1.0695x over previous
"""Ernie4 decoder layer (RMSNorm + GQA attention + shared expert + 16-expert
top-2 MoE) on 8 Trainium2 NeuronCores.

v2 design:
  - Attention head-parallel (2 q-heads + 1 kv-head per core), fp16 matmuls,
    causal block-skipping; o_proj partials combined with an fp16 ReduceScatter.
  - Router runs per-core on the fp32 post-attention x (own token block) BEFORE
    the AllGather; normalized top-2 weights are packed into the AllGather
    payload ([x fp16 | w fp16]) so selections are bit-identical across cores.
  - One AllGather of the packed payload; x^T built on-device (PE transposes)
    and kept resident in SBUF for router prefix-sums, shared expert and MoE.
  - Shared expert intermediate-sharded (256/core); its output initializes the
    combine buffer. Experts (2/core) gather tokens by indirect DMA, compute
    gate/up/down in fp16 with 1024-wide moving operands, scatter-add back.
  - Second fp16 ReduceScatter produces the final token-sharded output.
"""
import sys
sys.path.insert(0, "/opt/trn_rl_repo")

import numpy as np

import concourse.bass as bass
import concourse.bacc as bacc
import concourse.tile as tile
import concourse.mybir as mybir
from concourse import bass_utils

dt = mybir.dt
F32 = dt.float32
F32R = dt.float32r
F16 = dt.float16
I32 = dt.int32
AF = mybir.ActivationFunctionType
ALU = mybir.AluOpType
AX = mybir.AxisListType

T, H, NH, NKV, D = 1024, 2048, 16, 4, 128
E, I, IS = 16, 1024, 2048
ISC = IS // 8           # shared-expert intermediate per core
EPS = 1e-6
THETA = 10000.0
NCN = 8
P = 128
TB = T // P             # 8 token blocks
HC = H // P             # 16 hidden chunks
IP = I // P             # 8 expert-intermediate chunks
CAP = 256               # per-expert token capacity
PW = H + E              # AllGather payload width (x | w_topk)
BIG = 1.0e6
NEG = -30000.0          # fp16-safe mask value
RG = [list(range(NCN))]


def _emit(nc, tc):
    ex = {}
    for name, shape, d in [
        ("hid", [T, H], F32), ("hid_slice", [P, H], F32),
        ("w_qkv_pk", [P, HC * 512], F16),
        ("wo0", [D, H], F16), ("wo1", [D, H], F16),
        ("cosq", [D, T], F16), ("sinq", [D, T], F16),
        ("cosk", [D, T], F16), ("sink", [D, T], F16),
        ("permh", [P, P], F16), ("identh_in", [P, P], F16),
        ("identr_in", [P, P], F32), ("diagmask", [P, P], F16),
        ("gate_w_pk", [P, HC * E], F32), ("gate_b", [P, E], F32),
        ("emask0", [P, E], F32), ("emask1", [P, E], F32),
        ("ut_in", [P, P], F16), ("slb_in", [8, TB * P], F16),
        ("ws_g_pk", [P, HC * ISC], F16), ("ws_u_pk", [P, HC * ISC], F16),
        ("ws_d", [ISC, H], F16),
        ("we_g", [2, H, I], F16), ("we_u", [2, H, I], F16),
        ("we_d", [2, I, H], F16),
    ]:
        ex[name] = nc.dram_tensor(name, shape, d, kind="ExternalInput").ap()
    out_slice = nc.dram_tensor("out_slice", [P, H], F32, kind="ExternalOutput").ap()
    res_slice = nc.dram_tensor("res_slice", [P, H], F32, kind="ExternalOutput").ap()

    with tc.tile_pool(name="pp", bufs=1) as pp, \
         tc.tile_pool(name="dram", bufs=1, space="DRAM") as dram:
        rs_in = dram.tile([T, H], F16)
        rs_out = dram.tile([P, H], F16)
        ag_in = dram.tile([P, PW], F16)
        x_tm = dram.tile([T, PW], F16, addr_space="Shared")
        tok_lists = dram.tile([2 * CAP, 1], I32)
        rs2_in = dram.tile([T, H], F16)
        rs2_out = dram.tile([P, H], F16)

        identh = pp.tile([P, P], F16)
        nc.sync.dma_start(identh[:], ex["identh_in"][:])
        identf = pp.tile([P, P], F32)
        nc.sync.dma_start(identf[:], ex["identr_in"][:])
        eps_t = pp.tile([P, 1], F32)
        nc.vector.memset(eps_t[:], EPS)

        # ======== Phase A: norm + transpose + QKV + rope ========
        with tc.tile_pool(name="pab", bufs=1) as pab:
            qT = [pab.tile([P, T], F16, tag=f"qT{j}", name=f"qT{j}")
                  for j in range(2)]
            kT = pab.tile([P, T], F16)
            v_tm = pab.tile([P, TB * D], F16)
            wo_sb = [pab.tile([P, H], F16, tag=f"wo{j}", name=f"wo{j}")
                     for j in range(2)]
            diagm = pab.tile([P, P], F16)

            with tc.tile_pool(name="pa", bufs=1) as pa, \
                 tc.tile_pool(name="pa2", bufs=2) as pa2:
                # hid prefetch first so norm can start immediately
                hidbs = []
                for b in range(TB):
                    t_ = pa2.tile([P, H], F32, tag="hidb", bufs=8,
                                  name=f"hidb{b}")
                    nc.sync.dma_start(t_[:], ex["hid"][b * P:(b + 1) * P, :])
                    hidbs.append(t_)
                nc.sync.dma_start(wo_sb[0][:], ex["wo0"][:])
                nc.sync.dma_start(wo_sb[1][:], ex["wo1"][:])
                nc.sync.dma_start(diagm[:], ex["diagmask"][:])
                cosq = pa.tile([D, T], F16)
                sinq = pa.tile([D, T], F16)
                cosk = pa.tile([D, T], F16)
                sink = pa.tile([D, T], F16)
                for t_, s_ in [(cosq, "cosq"), (sinq, "sinq"),
                               (cosk, "cosk"), (sink, "sink")]:
                    nc.sync.dma_start(t_[:], ex[s_][:])
                permh = pa.tile([P, P], F16)
                nc.sync.dma_start(permh[:], ex["permh"][:])
                wqkv_sb = pa.tile([P, HC * 512], F16)
                nc.sync.dma_start(wqkv_sb[:], ex["w_qkv_pk"][:])

                x0T = [pa.tile([P, T], F16, tag=f"x0T{hc}", name=f"x0T{hc}")
                       for hc in range(HC)]
                qraw = [pa.tile([P, T], F16, tag=f"qraw{j}", name=f"qraw{j}")
                        for j in range(2)]
                kraw = pa.tile([P, T], F16)
                vraw = pa.tile([P, T], F16)
                dump = pa.tile([P, H], F32)

                with tc.tile_pool(name="psA1", bufs=2, space="PSUM") as psA1, \
                     tc.tile_pool(name="psA2", bufs=1, space="PSUM") as psA2:
                    for n in range(2):
                        x0hs = []
                        for bb in range(TB // 2):
                            b = n * (TB // 2) + bb
                            hidb = hidbs[b]
                            ssum = pa2.tile([P, 1], F32, tag="ssum")
                            nc.scalar.activation(dump[:], hidb[:], AF.Square,
                                                 accum_out=ssum[:, :1])
                            rms = pa2.tile([P, 1], F32, tag="rms")
                            nc.scalar.activation(rms[:], ssum[:], AF.Sqrt,
                                                 bias=eps_t[:, :1],
                                                 scale=1.0 / H)
                            inv = pa2.tile([P, 1], F32, tag="inv")
                            nc.vector.reciprocal(inv[:], rms[:])
                            x0h = pa2.tile([P, H], F16, tag="x0h", bufs=5,
                                           name=f"x0h{b}")
                            nc.vector.tensor_scalar_mul(x0h[:], hidb[:],
                                                        inv[:, :1])
                            x0hs.append(x0h)
                        sl = slice(n * 512, (n + 1) * 512)
                        for hc in range(HC):
                            tp = psA1.tile([P, 512], F16, tag="tpA")
                            for bb in range(4):
                                nc.tensor.transpose(
                                    tp[:, bb * P:(bb + 1) * P],
                                    x0hs[bb][:, hc * P:(hc + 1) * P],
                                    identh[:])
                            if hc % 2 == 0:
                                nc.vector.tensor_copy(x0T[hc][:, sl], tp[:])
                            else:
                                nc.scalar.activation(x0T[hc][:, sl], tp[:],
                                                     AF.Copy)
                        # QKV for this half of T
                        for j, (c0, dst) in enumerate(
                                [(0, qraw[0]), (128, qraw[1]),
                                 (256, kraw), (384, vraw)]):
                            ps = psA2.tile([P, 512], F32, tag=f"qkv{j}",
                                           name=f"qkv{j}")
                            for hc in range(HC):
                                nc.tensor.matmul(
                                    ps[:],
                                    wqkv_sb[:, hc * 512 + c0:hc * 512 + c0 + P],
                                    x0T[hc][:, sl],
                                    start=(hc == 0), stop=(hc == HC - 1))
                            nc.vector.tensor_copy(dst[:, sl], ps[:])

                # rope + v transpose
                with tc.tile_pool(name="psA3", bufs=2, space="PSUM") as psA3, \
                     tc.tile_pool(name="psA4", bufs=2, space="PSUM") as psA4:
                    for src, dst, c_, s_ in [(qraw[0], qT[0], cosq, sinq),
                                             (qraw[1], qT[1], cosq, sinq),
                                             (kraw, kT, cosk, sink)]:
                        sw = psA3.tile([P, T], F32, tag="sw")
                        for nn in range(2):
                            sl = slice(nn * 512, (nn + 1) * 512)
                            nc.tensor.matmul(sw[:, sl], permh[:], src[:, sl],
                                             start=True, stop=True)
                        t1 = pa2.tile([P, T], F16, tag="ropet1")
                        nc.vector.tensor_mul(t1[:], src[:], c_[:])
                        t2 = pa2.tile([P, T], F16, tag="ropet2")
                        nc.vector.tensor_mul(t2[:], sw[:], s_[:])
                        nc.vector.tensor_add(dst[:], t1[:], t2[:])
                    for g4 in range(2):
                        tp = psA4.tile([P, 512], F16, tag="tpV")
                        for bb in range(4):
                            b = g4 * 4 + bb
                            nc.tensor.transpose(
                                tp[:, bb * P:(bb + 1) * P],
                                vraw[:, b * P:(b + 1) * P], identh[:])
                        nc.vector.tensor_copy(
                            v_tm[:, g4 * 512:(g4 + 1) * 512], tp[:])

            # ======== Phase B: attention + o_proj ========
            with tc.tile_pool(name="pb", bufs=1) as pb, \
                 tc.tile_pool(name="pb2", bufs=2) as pb2:
                attnT = pb.tile([P, TB * T], F16)
                attnT3 = attnT[:].rearrange("p (kc t) -> p kc t", kc=TB)
                oT = [pb.tile([P, T], F16, tag=f"oT{j}", name=f"oT{j}")
                      for j in range(2)]
                with tc.tile_pool(name="psBs", bufs=2, space="PSUM") as psBs, \
                     tc.tile_pool(name="psBt", bufs=2, space="PSUM") as psBt, \
                     tc.tile_pool(name="psBo", bufs=1, space="PSUM") as psBo:
                    for h in range(2):
                        prev = None
                        for qc in range(TB + 1):
                            cur = None
                            if qc < TB:
                                W = (qc + 1) * P
                                sc = psBs.tile([P, T], F32, tag="sc")
                                for c0 in range(0, W, 512):
                                    c1 = min(c0 + 512, W)
                                    nc.tensor.matmul(
                                        sc[:, c0:c1],
                                        qT[h][:, qc * P:(qc + 1) * P],
                                        kT[:, c0:c1], start=True, stop=True)
                                nc.vector.tensor_tensor(
                                    out=sc[:, W - P:W], in0=sc[:, W - P:W],
                                    in1=diagm[:], op=ALU.add)
                                probs = pb2.tile([P, T], F16, tag="probs")
                                ssum = pb2.tile([P, 1], F32, tag="esum")
                                nc.scalar.activation(probs[:, :W], sc[:, :W],
                                                     AF.Exp,
                                                     accum_out=ssum[:, :1])
                                rec = pb2.tile([P, 1], F32, tag="rec")
                                nc.vector.reciprocal(rec[:], ssum[:])
                                nc.vector.tensor_scalar_mul(probs[:, :W],
                                                            probs[:, :W],
                                                            rec[:, :1])
                                cur = (probs, qc)
                            if prev is not None:
                                pprobs, pqc = prev
                                for g4 in range(0, pqc + 1, 4):
                                    cnt = min(4, pqc + 1 - g4)
                                    tp = psBt.tile([P, 512], F16, tag="tpB")
                                    for i in range(cnt):
                                        kc = g4 + i
                                        nc.tensor.transpose(
                                            tp[:, i * P:(i + 1) * P],
                                            pprobs[:, kc * P:(kc + 1) * P],
                                            identh[:])
                                    dst = attnT3[:, g4:g4 + cnt,
                                                 pqc * P:(pqc + 1) * P]
                                    src = tp[:, :cnt * P].rearrange(
                                        "p (kc t) -> p kc t", kc=cnt)
                                    if pqc % 2 == 0:
                                        nc.vector.tensor_copy(dst, src)
                                    else:
                                        nc.scalar.activation(dst, src,
                                                             AF.Copy)
                            prev = cur
                        ov = psBo.tile([P, T], F32, tag="ov")
                        for nn in range(2):
                            lo, hi = nn * 512, (nn + 1) * 512
                            kcs = [kc for kc in range(TB) if kc * P < hi]
                            for kc in kcs:
                                c0 = max(kc * P, lo)
                                nc.tensor.matmul(
                                    ov[:, c0:hi],
                                    v_tm[:, kc * P:(kc + 1) * P],
                                    attnT[:, kc * T + c0:kc * T + hi],
                                    start=(kc == 0),
                                    stop=(kc == kcs[-1]),
                                    skip_group_check=True)
                        nc.vector.tensor_copy(oT[h][:], ov[:])
                # o_proj
                with tc.tile_pool(name="psBp", bufs=2, space="PSUM") as psBp:
                    for tb_ in range(TB):
                        for nn in range(2):
                            sl = slice(nn * 1024, (nn + 1) * 1024)
                            ps = psBp.tile([P, 1024], F32, tag="psO")
                            for h in range(2):
                                for q2 in range(2):
                                    s2 = slice(q2 * 512, (q2 + 1) * 512)
                                    nc.tensor.matmul(
                                        ps[:, s2],
                                        oT[h][:, tb_ * P:(tb_ + 1) * P],
                                        wo_sb[h][:, nn * 1024 + q2 * 512:
                                                  nn * 1024 + (q2 + 1) * 512],
                                        start=(h == 0), stop=(h == 1))
                            ob = pb2.tile([P, 1024], F16, tag="ob")
                            nc.vector.tensor_copy(ob[:], ps[:])
                            nc.sync.dma_start(
                                rs_in[tb_ * P:(tb_ + 1) * P, sl], ob[:])

        # ======== weight prefetch pool (covers rest of kernel) ========
        with tc.tile_pool(name="pw", bufs=1) as pw:
            xT = [pw.tile([P, T], F16, tag=f"xT{hc}", name=f"xT{hc}")
                  for hc in range(HC)]
            wd_res = [pw.tile([P, H], F16, tag=f"wd{e}_{ip}",
                              name=f"wd{e}_{ip}")
                      for e in range(2) for ip in range(IP)]
            for e in range(2):
                for ip in range(IP):
                    nc.sync.dma_start(
                        wd_res[e * IP + ip][:],
                        ex["we_d"][e, ip * P:(ip + 1) * P, :])
            wsg_sb = pw.tile([P, HC * ISC], F16)
            nc.sync.dma_start(wsg_sb[:], ex["ws_g_pk"][:])
            wsu_sb = pw.tile([P, HC * ISC], F16)
            nc.sync.dma_start(wsu_sb[:], ex["ws_u_pk"][:])
            wsd_sb = [pw.tile([P, H], F16, tag=f"wsd{sp}", name=f"wsd{sp}")
                      for sp in range(2)]
            for sp in range(2):
                nc.sync.dma_start(wsd_sb[sp][:],
                                  ex["ws_d"][sp * P:(sp + 1) * P, :])
            gwr = pw.tile([P, HC * E], F32)
            nc.sync.dma_start(gwr[:], ex["gate_w_pk"][:])
            gate_b_sb = pw.tile([P, E], F32)
            nc.sync.dma_start(gate_b_sb[:], ex["gate_b"][:])
            ut_sb = pw.tile([P, P], F16)
            nc.sync.dma_start(ut_sb[:], ex["ut_in"][:])
            slb_sb = pw.tile([8, TB * P], F16)
            nc.sync.dma_start(slb_sb[:], ex["slb_in"][:])
            em = [pw.tile([P, E], F32, tag=f"em{e}", name=f"em{e}")
                  for e in range(2)]
            nc.sync.dma_start(em[0][:], ex["emask0"][:])
            nc.sync.dma_start(em[1][:], ex["emask1"][:])

            nc.gpsimd.collective_compute(
                "ReduceScatter", ALU.add, ins=[rs_in.opt()],
                outs=[rs_out.opt()], replica_groups=RG)

            # ======== Phase D: residual + norm + local router ========
            with tc.tile_pool(name="pd", bufs=1) as pd:
                hid_sl = pd.tile([P, H], F32)
                nc.sync.dma_start(hid_sl[:], ex["hid_slice"][:])
                attn_sl = pd.tile([P, H], F16)
                nc.sync.dma_start(attn_sl[:], rs_out[:])
                res_sb = pd.tile([P, H], F32)
                nc.vector.tensor_add(res_sb[:], hid_sl[:], attn_sl[:])
                nc.sync.dma_start(res_slice[:], res_sb[:])
                dump2 = pd.tile([P, H], F32)
                ssum = pd.tile([P, 1], F32)
                nc.scalar.activation(dump2[:], res_sb[:], AF.Square,
                                     accum_out=ssum[:, :1])
                rms = pd.tile([P, 1], F32)
                nc.scalar.activation(rms[:], ssum[:], AF.Sqrt,
                                     bias=eps_t[:, :1], scale=1.0 / H)
                inv = pd.tile([P, 1], F32)
                nc.vector.reciprocal(inv[:], rms[:])
                x_sl = pd.tile([P, H], F32)
                nc.vector.tensor_scalar_mul(x_sl[:], res_sb[:], inv[:, :1])
                pay = pd.tile([P, PW], F16)
                nc.vector.tensor_copy(pay[:, :H], x_sl[:])
                # local router on fp32 x
                xsT = pd.tile([P, HC * P], F32)
                with tc.tile_pool(name="psDt", bufs=2, space="PSUM") as psDt:
                    for g4 in range(4):
                        tp = psDt.tile([P, 512], F32, tag="tpD")
                        for i in range(4):
                            hc = g4 * 4 + i
                            nc.tensor.transpose(
                                tp[:, i * P:(i + 1) * P],
                                x_sl[:, hc * P:(hc + 1) * P], identf[:])
                        nc.vector.tensor_copy(
                            xsT[:, g4 * 512:(g4 + 1) * 512], tp[:])
                with tc.tile_pool(name="psDr", bufs=1, space="PSUM") as psDr:
                    lg = psDr.tile([P, E], F32, tag="lg")
                    for hc in range(HC):
                        nc.tensor.matmul(lg[:], xsT[:, hc * P:(hc + 1) * P],
                                         gwr[:, hc * E:(hc + 1) * E],
                                         start=(hc == 0), stop=(hc == HC - 1))
                    sig = pd.tile([P, E], F32)
                    nc.scalar.activation(sig[:], lg[:], AF.Sigmoid)
                sb_ = pd.tile([P, E], F32)
                nc.vector.tensor_add(sb_[:], sig[:], gate_b_sb[:])
                mx8 = pd.tile([P, 8], F32)
                nc.vector.max(out=mx8[:], in_=sb_[:])
                s1 = pd.tile([P, E], F32)
                nc.vector.tensor_tensor(out=s1[:], in0=sb_[:],
                                        in1=mx8[:, 0:1].to_broadcast([P, E]),
                                        op=ALU.is_equal)
                s2 = pd.tile([P, E], F32)
                nc.vector.tensor_tensor(out=s2[:], in0=sb_[:],
                                        in1=mx8[:, 1:2].to_broadcast([P, E]),
                                        op=ALU.is_equal)
                nc.vector.tensor_add(s1[:], s1[:], s2[:])
                nc.vector.tensor_scalar_min(s1[:], s1[:], 1.0)
                wa = pd.tile([P, E], F32)
                nc.vector.tensor_mul(wa[:], s1[:], sig[:])
                nrm = pd.tile([P, 1], F32)
                nc.vector.reduce_sum(nrm[:], wa[:], axis=AX.X)
                rec = pd.tile([P, 1], F32)
                nc.vector.reciprocal(rec[:], nrm[:])
                w_tm = pd.tile([P, E], F32)
                nc.vector.tensor_scalar_mul(w_tm[:], wa[:], rec[:, :1])
                nc.vector.tensor_copy(pay[:, H:], w_tm[:])
                nc.sync.dma_start(ag_in[:], pay[:])

            nc.gpsimd.collective_compute(
                "AllGather", ALU.bypass, ins=[ag_in.opt()],
                outs=[x_tm.opt()], replica_groups=RG)

            # ======== Phase X: build x^T in SBUF + global router ========
            with tc.tile_pool(name="px", bufs=1) as px, \
                 tc.tile_pool(name="px2", bufs=2) as px2:
                totals = px.tile([8, E], F16)
                grank = [px.tile([P, E], F32, tag=f"grank{b}",
                                 name=f"grank{b}") for b in range(TB)]
                selb = [px.tile([P, E], F16, tag=f"selb{b}",
                                name=f"selb{b}") for b in range(TB)]
                pre_sb = [px.tile([P, E], F16, tag=f"pre{b}",
                                  name=f"pre{b}") for b in range(TB)]
                with tc.tile_pool(name="psXt", bufs=2, space="PSUM") as psXt, \
                     tc.tile_pool(name="psXr", bufs=2, space="PSUM") as psXr:
                    xbs = []
                    for b in range(TB):
                        xb = px2.tile([P, PW], F16, tag="xb", bufs=8,
                                      name=f"xb{b}")
                        nc.sync.dma_start(xb[:],
                                          x_tm[b * P:(b + 1) * P, :])
                        xbs.append(xb)
                    for b in range(TB):
                        xb = xbs[b]
                        nc.vector.tensor_scalar(out=selb[b][:],
                                                in0=xb[:, H:PW],
                                                scalar1=0.0, scalar2=1.0,
                                                op0=ALU.is_gt, op1=ALU.mult)
                        pr = psXr.tile([P, E], F32, tag="pr")
                        nc.tensor.matmul(pr[:], ut_sb[:], selb[b][:],
                                         start=True, stop=True)
                        nc.vector.tensor_copy(pre_sb[b][:], pr[:])
                        nc.sync.dma_start(totals[b:b + 1, :],
                                          pre_sb[b][127:128, :])
                    for n in range(2):
                        sl = slice(n * 512, (n + 1) * 512)
                        for hc in range(HC):
                            tp = psXt.tile([P, 512], F16, tag="tpX")
                            for bb in range(4):
                                b = n * 4 + bb
                                nc.tensor.transpose(
                                    tp[:, bb * P:(bb + 1) * P],
                                    xbs[b][:, hc * P:(hc + 1) * P],
                                    identh[:])
                            if hc % 2 == 0:
                                nc.vector.tensor_copy(xT[hc][:, sl], tp[:])
                            else:
                                nc.scalar.activation(xT[hc][:, sl], tp[:],
                                                     AF.Copy)
                    for b in range(TB):
                        ofs = psXr.tile([P, E], F32, tag="ofs")
                        nc.tensor.matmul(ofs[:], slb_sb[:, b * P:(b + 1) * P],
                                         totals[:], start=True, stop=True)
                        nc.vector.tensor_tensor(out=grank[b][:],
                                                in0=pre_sb[b][:], in1=ofs[:],
                                                op=ALU.add)
                        gm = px2.tile([P, E], F32, tag="gm")
                        nc.vector.tensor_scalar(out=gm[:], in0=grank[b][:],
                                                scalar1=float(CAP),
                                                scalar2=BIG,
                                                op0=ALU.is_gt, op1=ALU.mult)
                        nc.vector.tensor_add(grank[b][:], grank[b][:], gm[:])
                        um = px2.tile([P, E], F32, tag="um")
                        nc.vector.tensor_scalar(out=um[:], in0=selb[b][:],
                                                scalar1=-BIG, scalar2=BIG,
                                                op0=ALU.mult, op1=ALU.add)
                        nc.vector.tensor_add(grank[b][:], grank[b][:], um[:])
                # token lists (grank is 1-based; -1 folded into scatter base)
                sent = px.tile([P, 1], I32)
                nc.vector.memset(sent[:], 1000000)
                for kk in range(2 * CAP // P):
                    nc.sync.dma_start(tok_lists[kk * P:(kk + 1) * P, :],
                                      sent[:])
                for b in range(TB):
                    tok = px2.tile([P, 1], I32, tag="tok")
                    nc.gpsimd.iota(tok[:], pattern=[[0, 1]], base=b * P,
                                   channel_multiplier=1)
                    for ei in range(2):
                        gsel = px2.tile([P, E], F32, tag="gsel")
                        nc.vector.tensor_mul(gsel[:], grank[b][:], em[ei][:])
                        ridx = px2.tile([P, 1], F32, tag="ridx")
                        nc.vector.reduce_sum(ridx[:], gsel[:], axis=AX.X)
                        nc.vector.tensor_scalar_add(ridx[:], ridx[:],
                                                    float(ei * CAP - 1))
                        ridx_i = px2.tile([P, 1], I32, tag="ridxi")
                        nc.vector.tensor_copy(ridx_i[:], ridx[:])
                        nc.gpsimd.indirect_dma_start(
                            out=tok_lists[:],
                            out_offset=bass.IndirectOffsetOnAxis(
                                ap=ridx_i[:, :1], axis=0),
                            in_=tok[:], in_offset=None,
                            bounds_check=2 * CAP - 1, oob_is_err=False)

            with tc.tile_pool(name="pg", bufs=1) as pg, \
                 tc.tile_pool(name="pg2", bufs=2) as pg2:
                idx_sb = [[pg.tile([P, 1], I32, tag=f"idx{ei}_{k}",
                                   name=f"idx{ei}_{k}") for k in range(2)]
                          for ei in range(2)]
                gxT = [pg.tile([P, HC * 2 * P], F16, tag=f"gxT{ei}",
                               name=f"gxT{ei}") for ei in range(2)]
                wcol = [[pg.tile([P, 1], F32, tag=f"wcol{ei}_{k}",
                                 name=f"wcol{ei}_{k}") for k in range(2)]
                        for ei in range(2)]

                # ======== Shared expert ========
                hs = [pg.tile([P, T], F16, tag=f"hs{sp}", name=f"hs{sp}")
                      for sp in range(2)]
                with tc.tile_pool(name="psS", bufs=1, space="PSUM") as psS:
                    gps = [psS.tile([P, T], F32, tag=f"sgp{sp}",
                                    name=f"sgp{sp}") for sp in range(2)]
                    ups = [psS.tile([P, T], F32, tag=f"sup{sp}",
                                    name=f"sup{sp}") for sp in range(2)]
                    for hc in range(HC):
                        for sp in range(2):
                            c0 = hc * ISC + sp * P
                            for nn in range(2):
                                sl = slice(nn * 512, (nn + 1) * 512)
                                nc.tensor.matmul(gps[sp][:, sl],
                                                 wsg_sb[:, c0:c0 + P],
                                                 xT[hc][:, sl],
                                                 start=(hc == 0),
                                                 stop=(hc == HC - 1))
                                nc.tensor.matmul(ups[sp][:, sl],
                                                 wsu_sb[:, c0:c0 + P],
                                                 xT[hc][:, sl],
                                                 start=(hc == 0),
                                                 stop=(hc == HC - 1))
                    for sp in range(2):
                        ga = pg2.tile([P, T], F16, tag="ga")
                        nc.scalar.activation(ga[:], gps[sp][:], AF.Silu)
                        nc.vector.tensor_mul(hs[sp][:], ga[:], ups[sp][:])
                with tc.tile_pool(name="psS3", bufs=2, space="PSUM") as psS3:
                    for nn in range(2):
                        for tb_ in range(TB):
                            ps3 = psS3.tile([P, 1024], F32, tag="psSd")
                            for sp in range(2):
                                for q2 in range(2):
                                    s2 = slice(q2 * 512, (q2 + 1) * 512)
                                    nc.tensor.matmul(
                                        ps3[:, s2],
                                        hs[sp][:, tb_ * P:(tb_ + 1) * P],
                                        wsd_sb[sp][:, nn * 1024 + q2 * 512:
                                                    nn * 1024 + (q2 + 1) * 512],
                                        start=(sp == 0), stop=(sp == 1))
                            sd = pg2.tile([P, 1024], F16, tag="sd")
                            nc.vector.tensor_copy(sd[:], ps3[:])
                            nc.sync.dma_start(
                                rs2_in[tb_ * P:(tb_ + 1) * P,
                                       nn * 1024:(nn + 1) * 1024], sd[:])

                # ======== Expert gathers (gpsimd overlaps shared) ========
                with tc.tile_pool(name="psFt", bufs=2, space="PSUM") as psFt:
                    for ei in range(2):
                        for k in range(2):
                            nc.sync.dma_start(
                                idx_sb[ei][k][:],
                                tok_lists[ei * CAP + k * P:
                                          ei * CAP + (k + 1) * P, :])
                            gx = pg2.tile([P, PW], F16, tag="gx")
                            nc.vector.memset(gx[:], 0.0)
                            nc.gpsimd.indirect_dma_start(
                                out=gx[:], out_offset=None,
                                in_=x_tm[:],
                                in_offset=bass.IndirectOffsetOnAxis(
                                    ap=idx_sb[ei][k][:, :1], axis=0),
                                bounds_check=T - 1, oob_is_err=False)
                            wtmp = pg2.tile([P, E], F32, tag="wtmp")
                            nc.vector.tensor_mul(wtmp[:], gx[:, H:PW],
                                                 em[ei][:])
                            nc.vector.reduce_sum(wcol[ei][k][:], wtmp[:],
                                                 axis=AX.X)
                            for g4 in range(4):
                                tp = psFt.tile([P, 512], F16, tag="tpF")
                                for i in range(4):
                                    hc = g4 * 4 + i
                                    nc.tensor.transpose(
                                        tp[:, i * P:(i + 1) * P],
                                        gx[:, hc * P:(hc + 1) * P],
                                        identh[:])
                                dst = gxT[ei][:, k * HC * P + g4 * 512:
                                              k * HC * P + (g4 + 1) * 512]
                                if g4 % 2 == 0:
                                    nc.vector.tensor_copy(dst, tp[:])
                                else:
                                    nc.scalar.activation(dst, tp[:], AF.Copy)

                # ======== Experts ========
                for ei in range(2):
                    with tc.tile_pool(name=f"pfw{ei}", bufs=1) as pfw, \
                         tc.tile_pool(name=f"psF1{ei}", bufs=1,
                                      space="PSUM") as psF1:
                        h_tm = [pg.tile([P, I], F16, tag=f"htm{k}",
                                        name=f"htm{ei}_{k}")
                                for k in range(2)]
                        g_ps = [psF1.tile([P, I], F32, tag=f"fg{k}",
                                          name=f"fg{k}_{ei}")
                                for k in range(2)]
                        u_ps = [psF1.tile([P, I], F32, tag=f"fu{k}",
                                          name=f"fu{k}_{ei}")
                                for k in range(2)]
                        for hc in range(HC):
                            wg = pfw.tile([P, I], F16, tag="wg", bufs=5)
                            nc.sync.dma_start(
                                wg[:], ex["we_g"][ei, hc * P:(hc + 1) * P, :])
                            wu = pfw.tile([P, I], F16, tag="wu", bufs=5)
                            nc.sync.dma_start(
                                wu[:], ex["we_u"][ei, hc * P:(hc + 1) * P, :])
                            for k in range(2):
                                c0 = k * HC * P + hc * P
                                for nn in range(2):
                                    sl = slice(nn * 512, (nn + 1) * 512)
                                    nc.tensor.matmul(g_ps[k][:, sl],
                                                     gxT[ei][:, c0:c0 + P],
                                                     wg[:, sl],
                                                     start=(hc == 0),
                                                     stop=(hc == HC - 1))
                                    nc.tensor.matmul(u_ps[k][:, sl],
                                                     gxT[ei][:, c0:c0 + P],
                                                     wu[:, sl],
                                                     start=(hc == 0),
                                                     stop=(hc == HC - 1))
                        for k in range(2):
                            sg = pg2.tile([P, I], F16, tag="sg")
                            nc.scalar.activation(sg[:], g_ps[k][:], AF.Silu)
                            nc.vector.tensor_mul(h_tm[k][:], sg[:],
                                                 u_ps[k][:])
                    h_sb = pg.tile([P, IP * 2 * P], F16, tag=f"hsb{ei}",
                                   name=f"hsb{ei}")
                    with tc.tile_pool(name=f"psF2{ei}", bufs=2,
                                      space="PSUM") as psF2:
                        for k in range(2):
                            for g4 in range(2):
                                tp = psF2.tile([P, 512], F16, tag="tpF2")
                                for i in range(4):
                                    ip = g4 * 4 + i
                                    nc.tensor.transpose(
                                        tp[:, i * P:(i + 1) * P],
                                        h_tm[k][:, ip * P:(ip + 1) * P],
                                        identh[:])
                                dst = h_sb[:, k * IP * P + g4 * 512:
                                           k * IP * P + (g4 + 1) * 512]
                                if g4 % 2 == 0:
                                    nc.vector.tensor_copy(dst, tp[:])
                                else:
                                    nc.scalar.activation(dst, tp[:],
                                                         AF.Copy)
                    with tc.tile_pool(name=f"psF3{ei}", bufs=2,
                                      space="PSUM") as psF3:
                        for k in range(2):
                            out_sb = pg2.tile([P, H], F16, tag="outsb",
                                              name=f"outsb{ei}{k}")
                            for nn in range(2):
                                sl = slice(nn * 1024, (nn + 1) * 1024)
                                psd = psF3.tile([P, 1024], F32, tag="fd")
                                for ip in range(IP):
                                    c0 = k * IP * P + ip * P
                                    for q2 in range(2):
                                        s2 = slice(q2 * 512, (q2 + 1) * 512)
                                        nc.tensor.matmul(
                                            psd[:, s2], h_sb[:, c0:c0 + P],
                                            wd_res[ei * IP + ip][
                                                :, nn * 1024 + q2 * 512:
                                                nn * 1024 + (q2 + 1) * 512],
                                            start=(ip == 0),
                                            stop=(ip == IP - 1))
                                nc.vector.tensor_scalar_mul(
                                    out_sb[:, sl], psd[:],
                                    wcol[ei][k][:, :1])
                            nc.gpsimd.indirect_dma_start(
                                out=rs2_in[:],
                                out_offset=bass.IndirectOffsetOnAxis(
                                    ap=idx_sb[ei][k][:, :1], axis=0),
                                in_=out_sb[:], in_offset=None,
                                bounds_check=T - 1, oob_is_err=False,
                                compute_op=ALU.add)

            nc.gpsimd.collective_compute(
                "ReduceScatter", ALU.add, ins=[rs2_in.opt()],
                outs=[rs2_out.opt()], replica_groups=RG)
            with tc.tile_pool(name="pz", bufs=1) as pz:
                fin = pz.tile([P, H], F16)
                nc.sync.dma_start(fin[:], rs2_out[:])
                fin32 = pz.tile([P, H], F32)
                nc.vector.tensor_copy(fin32[:], fin[:])
                nc.sync.dma_start(out_slice[:], fin32[:])


_CACHE = {}


def _build():
    key = "nc"
    if key in _CACHE:
        return _CACHE[key]
    nc = bacc.Bacc("TRN2", target_bir_lowering=False, debug=False,
                   num_devices=NCN)
    with tile.TileContext(nc) as tc:
        _emit(nc, tc)
    nc.compile()
    _CACHE[key] = nc
    return nc


def _host_prep(inputs):
    f16 = np.float16
    pos = np.asarray(inputs["positions"]).astype(np.float64)
    hid = np.asarray(inputs["hidden_states"], np.float32)
    w_in = np.asarray(inputs["w_in_ln"], np.float32)
    w_post = np.asarray(inputs["w_post_ln"], np.float32)
    wq = np.asarray(inputs["wq"], np.float32) * w_in[:, None]
    wk = np.asarray(inputs["wk"], np.float32) * w_in[:, None]
    wv = np.asarray(inputs["wv"], np.float32) * w_in[:, None]
    wo = np.asarray(inputs["wo"], np.float32)
    gate_w = np.asarray(inputs["gate_w"], np.float32) * w_post[None, :]
    gate_b = np.asarray(inputs["gate_bias"], np.float32).reshape(1, E)
    we_g = (np.asarray(inputs["we_gate"], np.float32)
            * w_post[None, :, None]).astype(f16)
    we_u = (np.asarray(inputs["we_up"], np.float32)
            * w_post[None, :, None]).astype(f16)
    we_d = np.asarray(inputs["we_down"], np.float32).astype(f16)
    ws_g = np.asarray(inputs["ws_gate"], np.float32) * w_post[:, None]
    ws_u = np.asarray(inputs["ws_up"], np.float32) * w_post[:, None]
    ws_d = np.asarray(inputs["ws_down"], np.float32).astype(f16)

    inv_freq = 1.0 / (THETA ** (np.arange(0, D, 2, dtype=np.float64) / D))
    f = pos[None, :] * inv_freq[:, None]
    cos2, sin2 = np.cos(f), np.sin(f)
    cosT = np.repeat(cos2, 2, axis=0).astype(np.float32)
    sinT = np.empty((D, T), np.float32)
    sinT[0::2] = -sin2
    sinT[1::2] = sin2
    s = 1.0 / np.sqrt(D)
    cosq, sinq = (cosT * s).astype(f16), (sinT * s).astype(f16)
    cosk, sink = cosT.astype(f16), sinT.astype(f16)

    ii = np.arange(P)
    diagmask = np.where(ii[:, None] >= ii[None, :], 0.0, NEG).astype(f16)
    ident = np.eye(P, dtype=np.float32)
    ut_in = np.triu(np.ones((P, P), np.float32)).astype(f16)
    slb_in = np.zeros((8, TB * P), np.float32)
    for b in range(TB):
        slb_in[:b, b * P:(b + 1) * P] = 1.0
    slb_in = slb_in.astype(f16)
    perm = np.zeros((P, P), np.float32)
    for i in range(0, P, 2):
        perm[i, i + 1] = 1.0
        perm[i + 1, i] = 1.0

    # packed (chunk-major) stationary layouts: [128, HC*width]
    def pack_pk(w, width):  # w: [H, width]
        return np.ascontiguousarray(
            w.reshape(HC, P, width).transpose(1, 0, 2).reshape(P, HC * width))

    gate_w_pk = pack_pk(gate_w.T.astype(np.float32), E)

    maps = []
    for c in range(NCN):
        g = c // 2
        w_qkv = pack_pk(np.concatenate([
            wq[:, 2 * c * D:(2 * c + 1) * D],
            wq[:, (2 * c + 1) * D:(2 * c + 2) * D],
            wk[:, g * D:(g + 1) * D],
            wv[:, g * D:(g + 1) * D]], axis=1), 512).astype(f16)
        em0 = np.zeros((P, E), np.float32)
        em0[:, 2 * c] = 1.0
        em1 = np.zeros((P, E), np.float32)
        em1[:, 2 * c + 1] = 1.0
        maps.append({
            "hid": hid,
            "hid_slice": np.ascontiguousarray(hid[c * P:(c + 1) * P]),
            "w_qkv_pk": w_qkv,
            "wo0": np.ascontiguousarray(wo[2 * c * D:(2 * c + 1) * D]).astype(f16),
            "wo1": np.ascontiguousarray(
                wo[(2 * c + 1) * D:(2 * c + 2) * D]).astype(f16),
            "cosq": cosq, "sinq": sinq, "cosk": cosk, "sink": sink,
            "permh": perm.astype(f16), "identh_in": ident.astype(f16),
            "identr_in": ident, "diagmask": diagmask,
            "gate_w_pk": gate_w_pk,
            "gate_b": np.broadcast_to(gate_b, (P, E)).astype(np.float32).copy(),
            "emask0": em0, "emask1": em1,
            "ut_in": ut_in, "slb_in": slb_in,
            "ws_g_pk": pack_pk(
                ws_g[:, c * ISC:(c + 1) * ISC].astype(np.float32), ISC
            ).astype(f16),
            "ws_u_pk": pack_pk(
                ws_u[:, c * ISC:(c + 1) * ISC].astype(np.float32), ISC
            ).astype(f16),
            "ws_d": np.ascontiguousarray(ws_d[c * ISC:(c + 1) * ISC]),
            "we_g": np.ascontiguousarray(we_g[2 * c:2 * c + 2]),
            "we_u": np.ascontiguousarray(we_u[2 * c:2 * c + 2]),
            "we_d": np.ascontiguousarray(we_d[2 * c:2 * c + 2]),
        })
    return maps


def kernel(trace=False, **inputs):
    nc = _build()
    maps = _host_prep(inputs)
    res = bass_utils.run_bass_kernel_spmd(
        nc, maps, core_ids=list(range(NCN)), trace=trace)
    out = np.concatenate([res.results[c]["out_slice"] for c in range(NCN)], 0)
    resid = np.concatenate([res.results[c]["res_slice"] for c in range(NCN)], 0)
    kernel.last_results = res
    return out, resid


# revision 42
# speedup vs baseline: 1.0901x; 1.0193x over previous
"""Ernie4 decoder layer (RMSNorm + GQA attention + shared expert + 16-expert
top-2 MoE) on 8 Trainium2 NeuronCores.

v2 design:
  - Attention head-parallel (2 q-heads + 1 kv-head per core), fp16 matmuls,
    causal block-skipping; o_proj partials combined with an fp16 ReduceScatter.
  - Router runs per-core on the fp32 post-attention x (own token block) BEFORE
    the AllGather; normalized top-2 weights are packed into the AllGather
    payload ([x fp16 | w fp16]) so selections are bit-identical across cores.
  - One AllGather of the packed payload; x^T built on-device (PE transposes)
    and kept resident in SBUF for router prefix-sums, shared expert and MoE.
  - Shared expert intermediate-sharded (256/core); its output initializes the
    combine buffer. Experts (2/core) gather tokens by indirect DMA, compute
    gate/up/down in fp16 with 1024-wide moving operands, scatter-add back.
  - Second fp16 ReduceScatter produces the final token-sharded output.
"""
import sys
sys.path.insert(0, "/opt/trn_rl_repo")

import numpy as np

import concourse.bass as bass
import concourse.bacc as bacc
import concourse.tile as tile
import concourse.mybir as mybir
from concourse import bass_utils

dt = mybir.dt
F32 = dt.float32
F32R = dt.float32r
F16 = dt.float16
I32 = dt.int32
AF = mybir.ActivationFunctionType
ALU = mybir.AluOpType
AX = mybir.AxisListType

T, H, NH, NKV, D = 1024, 2048, 16, 4, 128
E, I, IS = 16, 1024, 2048
ISC = IS // 8           # shared-expert intermediate per core
EPS = 1e-6
THETA = 10000.0
NCN = 8
P = 128
TB = T // P             # 8 token blocks
HC = H // P             # 16 hidden chunks
IP = I // P             # 8 expert-intermediate chunks
CAP = 256               # per-expert token capacity
PW = H + E              # AllGather payload width (x | w_topk)
BIG = 1.0e6
NEG = -30000.0          # fp16-safe mask value
RG = [list(range(NCN))]


def _emit(nc, tc):
    ex = {}
    for name, shape, d in [
        ("hid", [T, H], F32), ("hid_slice", [P, H], F32),
        ("w_qkv_pk", [P, HC * 512], F16),
        ("wo0", [D, H], F16), ("wo1", [D, H], F16),
        ("cosq", [D, T], F16), ("sinq", [D, T], F16),
        ("cosk", [D, T], F16), ("sink", [D, T], F16),
        ("permh", [P, P], F16), ("identh_in", [P, P], F16),
        ("identr_in", [P, P], F32), ("diagmask", [P, P], F16),
        ("gate_w_pk", [P, HC * E], F32), ("gate_b", [P, E], F32),
        ("emask0", [P, E], F32), ("emask1", [P, E], F32),
        ("ut_in", [P, P], F16), ("slb_in", [8, TB * P], F16),
        ("ws_g_pk", [P, HC * ISC], F16), ("ws_u_pk", [P, HC * ISC], F16),
        ("ws_d", [ISC, H], F16),
        ("we_g", [2, H, I], F16), ("we_u", [2, H, I], F16),
        ("we_d", [2, I, H], F16),
    ]:
        ex[name] = nc.dram_tensor(name, shape, d, kind="ExternalInput").ap()
    out_slice = nc.dram_tensor("out_slice", [P, H], F32, kind="ExternalOutput").ap()
    res_slice = nc.dram_tensor("res_slice", [P, H], F32, kind="ExternalOutput").ap()

    with tc.tile_pool(name="pp", bufs=1) as pp, \
         tc.tile_pool(name="dram", bufs=1, space="DRAM") as dram:
        rs_in = dram.tile([T, H], F16)
        rs_out = dram.tile([P, H], F16)
        ag_in = dram.tile([P, PW], F16)
        x_tm = dram.tile([T, PW], F16, addr_space="Shared")
        tok_lists = dram.tile([2 * CAP, 1], I32)
        rs2_in = dram.tile([T, H], F16)
        rs2_out = dram.tile([P, H], F16)

        identh = pp.tile([P, P], F16)
        nc.sync.dma_start(identh[:], ex["identh_in"][:])
        identf = pp.tile([P, P], F32)
        nc.sync.dma_start(identf[:], ex["identr_in"][:])
        eps_t = pp.tile([P, 1], F32)
        nc.vector.memset(eps_t[:], EPS)

        # ======== Phase A: norm + transpose + QKV + rope ========
        with tc.tile_pool(name="pab", bufs=1) as pab:
            qT = [pab.tile([P, T], F16, tag=f"qT{j}", name=f"qT{j}")
                  for j in range(2)]
            kT = pab.tile([P, T], F16)
            v_tm = pab.tile([P, TB * D], F16)
            wo_sb = [pab.tile([P, H], F16, tag=f"wo{j}", name=f"wo{j}")
                     for j in range(2)]
            diagm = pab.tile([P, P], F16)

            with tc.tile_pool(name="pa", bufs=1) as pa, \
                 tc.tile_pool(name="pa2", bufs=2) as pa2:
                # hid prefetch first so norm can start immediately
                hidbs = []
                for b in range(TB):
                    t_ = pa2.tile([P, H], F32, tag="hidb", bufs=8,
                                  name=f"hidb{b}")
                    nc.sync.dma_start(t_[:], ex["hid"][b * P:(b + 1) * P, :])
                    hidbs.append(t_)
                nc.sync.dma_start(wo_sb[0][:], ex["wo0"][:])
                nc.sync.dma_start(wo_sb[1][:], ex["wo1"][:])
                nc.sync.dma_start(diagm[:], ex["diagmask"][:])
                cosq = pa.tile([D, T], F16)
                sinq = pa.tile([D, T], F16)
                cosk = pa.tile([D, T], F16)
                sink = pa.tile([D, T], F16)
                for t_, s_ in [(cosq, "cosq"), (sinq, "sinq"),
                               (cosk, "cosk"), (sink, "sink")]:
                    nc.sync.dma_start(t_[:], ex[s_][:])
                permh = pa.tile([P, P], F16)
                nc.sync.dma_start(permh[:], ex["permh"][:])
                wqkv_sb = pa.tile([P, HC * 512], F16)
                nc.sync.dma_start(wqkv_sb[:], ex["w_qkv_pk"][:])

                x0T = [pa.tile([P, T], F16, tag=f"x0T{hc}", name=f"x0T{hc}")
                       for hc in range(HC)]
                qraw = [pa.tile([P, T], F16, tag=f"qraw{j}", name=f"qraw{j}")
                        for j in range(2)]
                kraw = pa.tile([P, T], F16)
                vraw = pa.tile([P, T], F16)
                dump = pa.tile([P, H], F32)

                with tc.tile_pool(name="psA1", bufs=2, space="PSUM") as psA1, \
                     tc.tile_pool(name="psA2", bufs=1, space="PSUM") as psA2:
                    for n in range(2):
                        x0hs = []
                        for bb in range(TB // 2):
                            b = n * (TB // 2) + bb
                            hidb = hidbs[b]
                            ssum = pa2.tile([P, 1], F32, tag="ssum")
                            nc.scalar.activation(dump[:], hidb[:], AF.Square,
                                                 accum_out=ssum[:, :1])
                            rms = pa2.tile([P, 1], F32, tag="rms")
                            nc.scalar.activation(rms[:], ssum[:], AF.Sqrt,
                                                 bias=eps_t[:, :1],
                                                 scale=1.0 / H)
                            inv = pa2.tile([P, 1], F32, tag="inv")
                            nc.vector.reciprocal(inv[:], rms[:])
                            x0h = pa2.tile([P, H], F16, tag="x0h", bufs=5,
                                           name=f"x0h{b}")
                            nc.vector.tensor_scalar_mul(x0h[:], hidb[:],
                                                        inv[:, :1])
                            x0hs.append(x0h)
                        sl = slice(n * 512, (n + 1) * 512)
                        for hc in range(HC):
                            tp = psA1.tile([P, 512], F16, tag="tpA")
                            for bb in range(4):
                                nc.tensor.transpose(
                                    tp[:, bb * P:(bb + 1) * P],
                                    x0hs[bb][:, hc * P:(hc + 1) * P],
                                    identh[:])
                            if hc % 2 == 0:
                                nc.vector.tensor_copy(x0T[hc][:, sl], tp[:])
                            else:
                                nc.scalar.activation(x0T[hc][:, sl], tp[:],
                                                     AF.Copy)
                        # QKV for this half of T
                        for j, (c0, dst) in enumerate(
                                [(0, qraw[0]), (128, qraw[1]),
                                 (256, kraw), (384, vraw)]):
                            ps = psA2.tile([P, 512], F32, tag=f"qkv{j}",
                                           name=f"qkv{j}")
                            for hc in range(HC):
                                nc.tensor.matmul(
                                    ps[:],
                                    wqkv_sb[:, hc * 512 + c0:hc * 512 + c0 + P],
                                    x0T[hc][:, sl],
                                    start=(hc == 0), stop=(hc == HC - 1))
                            nc.vector.tensor_copy(dst[:, sl], ps[:])

                # rope + v transpose
                with tc.tile_pool(name="psA3", bufs=2, space="PSUM") as psA3, \
                     tc.tile_pool(name="psA4", bufs=2, space="PSUM") as psA4:
                    for src, dst, c_, s_ in [(qraw[0], qT[0], cosq, sinq),
                                             (qraw[1], qT[1], cosq, sinq),
                                             (kraw, kT, cosk, sink)]:
                        sw = psA3.tile([P, T], F32, tag="sw")
                        for nn in range(2):
                            sl = slice(nn * 512, (nn + 1) * 512)
                            nc.tensor.matmul(sw[:, sl], permh[:], src[:, sl],
                                             start=True, stop=True)
                        t1 = pa2.tile([P, T], F16, tag="ropet1")
                        nc.vector.tensor_mul(t1[:], src[:], c_[:])
                        t2 = pa2.tile([P, T], F16, tag="ropet2")
                        nc.vector.tensor_mul(t2[:], sw[:], s_[:])
                        nc.vector.tensor_add(dst[:], t1[:], t2[:])
                    for g4 in range(2):
                        tp = psA4.tile([P, 512], F16, tag="tpV")
                        for bb in range(4):
                            b = g4 * 4 + bb
                            nc.tensor.transpose(
                                tp[:, bb * P:(bb + 1) * P],
                                vraw[:, b * P:(b + 1) * P], identh[:])
                        nc.vector.tensor_copy(
                            v_tm[:, g4 * 512:(g4 + 1) * 512], tp[:])

            # ======== Phase B: attention + o_proj ========
            with tc.tile_pool(name="pb", bufs=1) as pb, \
                 tc.tile_pool(name="pb2", bufs=2) as pb2:
                attnT = pb.tile([P, TB * T], F16)
                attnT3 = attnT[:].rearrange("p (kc t) -> p kc t", kc=TB)
                oT = [pb.tile([P, T], F16, tag=f"oT{j}", name=f"oT{j}")
                      for j in range(2)]
                with tc.tile_pool(name="psBs", bufs=2, space="PSUM") as psBs, \
                     tc.tile_pool(name="psBt", bufs=2, space="PSUM") as psBt, \
                     tc.tile_pool(name="psBo", bufs=1, space="PSUM") as psBo:
                    for h in range(2):
                        prev = None
                        for qc in range(TB + 1):
                            cur = None
                            if qc < TB:
                                W = (qc + 1) * P
                                sc = psBs.tile([P, T], F32, tag="sc")
                                for c0 in range(0, W, 512):
                                    c1 = min(c0 + 512, W)
                                    nc.tensor.matmul(
                                        sc[:, c0:c1],
                                        qT[h][:, qc * P:(qc + 1) * P],
                                        kT[:, c0:c1], start=True, stop=True)
                                nc.vector.tensor_tensor(
                                    out=sc[:, W - P:W], in0=sc[:, W - P:W],
                                    in1=diagm[:], op=ALU.add)
                                probs = pb2.tile([P, T], F16, tag="probs")
                                ssum = pb2.tile([P, 1], F32, tag="esum")
                                nc.scalar.activation(probs[:, :W], sc[:, :W],
                                                     AF.Exp,
                                                     accum_out=ssum[:, :1])
                                rec = pb2.tile([P, 1], F32, tag="rec")
                                nc.vector.reciprocal(rec[:], ssum[:])
                                nc.vector.tensor_scalar_mul(probs[:, :W],
                                                            probs[:, :W],
                                                            rec[:, :1])
                                cur = (probs, qc)
                            if prev is not None:
                                pprobs, pqc = prev
                                for g4 in range(0, pqc + 1, 4):
                                    cnt = min(4, pqc + 1 - g4)
                                    tp = psBt.tile([P, 512], F16, tag="tpB")
                                    for i in range(cnt):
                                        kc = g4 + i
                                        nc.tensor.transpose(
                                            tp[:, i * P:(i + 1) * P],
                                            pprobs[:, kc * P:(kc + 1) * P],
                                            identh[:])
                                    dst = attnT3[:, g4:g4 + cnt,
                                                 pqc * P:(pqc + 1) * P]
                                    src = tp[:, :cnt * P].rearrange(
                                        "p (kc t) -> p kc t", kc=cnt)
                                    if pqc % 2 == 0:
                                        nc.vector.tensor_copy(dst, src)
                                    else:
                                        nc.scalar.activation(dst, src,
                                                             AF.Copy)
                            prev = cur
                        ov = psBo.tile([P, T], F32, tag="ov")
                        for nn in range(2):
                            lo, hi = nn * 512, (nn + 1) * 512
                            kcs = [kc for kc in range(TB) if kc * P < hi]
                            for kc in kcs:
                                c0 = max(kc * P, lo)
                                nc.tensor.matmul(
                                    ov[:, c0:hi],
                                    v_tm[:, kc * P:(kc + 1) * P],
                                    attnT[:, kc * T + c0:kc * T + hi],
                                    start=(kc == 0),
                                    stop=(kc == kcs[-1]),
                                    skip_group_check=True)
                        nc.vector.tensor_copy(oT[h][:], ov[:])
                # o_proj
                with tc.tile_pool(name="psBp", bufs=2, space="PSUM") as psBp:
                    for tb_ in range(TB):
                        for nn in range(2):
                            sl = slice(nn * 1024, (nn + 1) * 1024)
                            ps = psBp.tile([P, 1024], F32, tag="psO")
                            for h in range(2):
                                for q2 in range(2):
                                    s2 = slice(q2 * 512, (q2 + 1) * 512)
                                    nc.tensor.matmul(
                                        ps[:, s2],
                                        oT[h][:, tb_ * P:(tb_ + 1) * P],
                                        wo_sb[h][:, nn * 1024 + q2 * 512:
                                                  nn * 1024 + (q2 + 1) * 512],
                                        start=(h == 0), stop=(h == 1))
                            ob = pb2.tile([P, 1024], F16, tag="ob")
                            nc.vector.tensor_copy(ob[:], ps[:])
                            nc.sync.dma_start(
                                rs_in[tb_ * P:(tb_ + 1) * P, sl], ob[:])

        # ======== weight prefetch pool (covers rest of kernel) ========
        with tc.tile_pool(name="pw", bufs=1) as pw, \
             tc.tile_pool(name="pfw", bufs=1) as pfw:
            xT = [pw.tile([P, T], F16, tag=f"xT{hc}", name=f"xT{hc}")
                  for hc in range(HC)]
            wd_res = [pw.tile([P, H], F16, tag=f"wd{e}_{ip}",
                              name=f"wd{e}_{ip}")
                      for e in range(2) for ip in range(IP)]
            for e in range(2):
                for ip in range(IP):
                    nc.sync.dma_start(
                        wd_res[e * IP + ip][:],
                        ex["we_d"][e, ip * P:(ip + 1) * P, :])
            wsg_sb = pw.tile([P, HC * ISC], F16)
            nc.sync.dma_start(wsg_sb[:], ex["ws_g_pk"][:])
            wsu_sb = pw.tile([P, HC * ISC], F16)
            nc.sync.dma_start(wsu_sb[:], ex["ws_u_pk"][:])
            wsd_sb = [pw.tile([P, H], F16, tag=f"wsd{sp}", name=f"wsd{sp}")
                      for sp in range(2)]
            for sp in range(2):
                nc.sync.dma_start(wsd_sb[sp][:],
                                  ex["ws_d"][sp * P:(sp + 1) * P, :])
            gwr = pw.tile([P, HC * E], F32)
            nc.sync.dma_start(gwr[:], ex["gate_w_pk"][:])
            gate_b_sb = pw.tile([P, E], F32)
            nc.sync.dma_start(gate_b_sb[:], ex["gate_b"][:])
            ut_sb = pw.tile([P, P], F16)
            nc.sync.dma_start(ut_sb[:], ex["ut_in"][:])
            slb_sb = pw.tile([8, TB * P], F16)
            nc.sync.dma_start(slb_sb[:], ex["slb_in"][:])
            em = [pw.tile([P, E], F32, tag=f"em{e}", name=f"em{e}")
                  for e in range(2)]
            nc.sync.dma_start(em[0][:], ex["emask0"][:])
            nc.sync.dma_start(em[1][:], ex["emask1"][:])
            # pre-issue expert-0's first gate/up weight chunks
            wgl, wul = [], []
            for hc in range(5):
                wgp = pfw.tile([P, I], F16, tag="wg", bufs=5,
                               name=f"wgp{hc}")
                nc.sync.dma_start(wgp[:],
                                  ex["we_g"][0, hc * P:(hc + 1) * P, :])
                wgl.append(wgp)
                wup = pfw.tile([P, I], F16, tag="wu", bufs=5,
                               name=f"wup{hc}")
                nc.sync.dma_start(wup[:],
                                  ex["we_u"][0, hc * P:(hc + 1) * P, :])
                wul.append(wup)

            nc.gpsimd.collective_compute(
                "ReduceScatter", ALU.add, ins=[rs_in.opt()],
                outs=[rs_out.opt()], replica_groups=RG)

            # ======== Phase D: residual + norm + local router ========
            with tc.tile_pool(name="pd", bufs=1) as pd:
                hid_sl = pd.tile([P, H], F32)
                nc.sync.dma_start(hid_sl[:], ex["hid_slice"][:])
                attn_sl = pd.tile([P, H], F16)
                nc.sync.dma_start(attn_sl[:], rs_out[:])
                res_sb = pd.tile([P, H], F32)
                nc.vector.tensor_add(res_sb[:], hid_sl[:], attn_sl[:])
                nc.sync.dma_start(res_slice[:], res_sb[:])
                dump2 = pd.tile([P, H], F32)
                ssum = pd.tile([P, 1], F32)
                nc.scalar.activation(dump2[:], res_sb[:], AF.Square,
                                     accum_out=ssum[:, :1])
                rms = pd.tile([P, 1], F32)
                nc.scalar.activation(rms[:], ssum[:], AF.Sqrt,
                                     bias=eps_t[:, :1], scale=1.0 / H)
                inv = pd.tile([P, 1], F32)
                nc.vector.reciprocal(inv[:], rms[:])
                x_sl = pd.tile([P, H], F32)
                nc.vector.tensor_scalar_mul(x_sl[:], res_sb[:], inv[:, :1])
                pay = pd.tile([P, PW], F16)
                nc.vector.tensor_copy(pay[:, :H], x_sl[:])
                # local router on fp32 x
                xsT = pd.tile([P, HC * P], F32)
                with tc.tile_pool(name="psDt", bufs=2, space="PSUM") as psDt:
                    for g4 in range(4):
                        tp = psDt.tile([P, 512], F32, tag="tpD")
                        for i in range(4):
                            hc = g4 * 4 + i
                            nc.tensor.transpose(
                                tp[:, i * P:(i + 1) * P],
                                x_sl[:, hc * P:(hc + 1) * P], identf[:])
                        nc.vector.tensor_copy(
                            xsT[:, g4 * 512:(g4 + 1) * 512], tp[:])
                with tc.tile_pool(name="psDr", bufs=1, space="PSUM") as psDr:
                    lg = psDr.tile([P, E], F32, tag="lg")
                    for hc in range(HC):
                        nc.tensor.matmul(lg[:], xsT[:, hc * P:(hc + 1) * P],
                                         gwr[:, hc * E:(hc + 1) * E],
                                         start=(hc == 0), stop=(hc == HC - 1))
                    sig = pd.tile([P, E], F32)
                    nc.scalar.activation(sig[:], lg[:], AF.Sigmoid)
                sb_ = pd.tile([P, E], F32)
                nc.vector.tensor_add(sb_[:], sig[:], gate_b_sb[:])
                mx8 = pd.tile([P, 8], F32)
                nc.vector.max(out=mx8[:], in_=sb_[:])
                s1 = pd.tile([P, E], F32)
                nc.vector.tensor_tensor(out=s1[:], in0=sb_[:],
                                        in1=mx8[:, 0:1].to_broadcast([P, E]),
                                        op=ALU.is_equal)
                s2 = pd.tile([P, E], F32)
                nc.vector.tensor_tensor(out=s2[:], in0=sb_[:],
                                        in1=mx8[:, 1:2].to_broadcast([P, E]),
                                        op=ALU.is_equal)
                nc.vector.tensor_add(s1[:], s1[:], s2[:])
                nc.vector.tensor_scalar_min(s1[:], s1[:], 1.0)
                wa = pd.tile([P, E], F32)
                nc.vector.tensor_mul(wa[:], s1[:], sig[:])
                nrm = pd.tile([P, 1], F32)
                nc.vector.reduce_sum(nrm[:], wa[:], axis=AX.X)
                rec = pd.tile([P, 1], F32)
                nc.vector.reciprocal(rec[:], nrm[:])
                w_tm = pd.tile([P, E], F32)
                nc.vector.tensor_scalar_mul(w_tm[:], wa[:], rec[:, :1])
                nc.vector.tensor_copy(pay[:, H:], w_tm[:])
                nc.sync.dma_start(ag_in[:], pay[:])

            nc.gpsimd.collective_compute(
                "AllGather", ALU.bypass, ins=[ag_in.opt()],
                outs=[x_tm.opt()], replica_groups=RG)

            # ======== Phase X: build x^T in SBUF + global router ========
            with tc.tile_pool(name="px", bufs=1) as px, \
                 tc.tile_pool(name="px2", bufs=2) as px2:
                totals = px.tile([8, E], F16)
                grank = [px.tile([P, E], F32, tag=f"grank{b}",
                                 name=f"grank{b}") for b in range(TB)]
                selb = [px.tile([P, E], F16, tag=f"selb{b}",
                                name=f"selb{b}") for b in range(TB)]
                pre_sb = [px.tile([P, E], F16, tag=f"pre{b}",
                                  name=f"pre{b}") for b in range(TB)]
                with tc.tile_pool(name="psXt", bufs=2, space="PSUM") as psXt, \
                     tc.tile_pool(name="psXr", bufs=2, space="PSUM") as psXr:
                    xbs = []
                    for b in range(TB):
                        xb = px2.tile([P, PW], F16, tag="xb", bufs=8,
                                      name=f"xb{b}")
                        nc.sync.dma_start(xb[:],
                                          x_tm[b * P:(b + 1) * P, :])
                        xbs.append(xb)
                    for b in range(TB):
                        xb = xbs[b]
                        nc.vector.tensor_scalar(out=selb[b][:],
                                                in0=xb[:, H:PW],
                                                scalar1=0.0, scalar2=1.0,
                                                op0=ALU.is_gt, op1=ALU.mult)
                        pr = psXr.tile([P, E], F32, tag="pr")
                        nc.tensor.matmul(pr[:], ut_sb[:], selb[b][:],
                                         start=True, stop=True)
                        nc.vector.tensor_copy(pre_sb[b][:], pr[:])
                        nc.sync.dma_start(totals[b:b + 1, :],
                                          pre_sb[b][127:128, :])
                    for n in range(2):
                        sl = slice(n * 512, (n + 1) * 512)
                        for hc in range(HC):
                            tp = psXt.tile([P, 512], F16, tag="tpX")
                            for bb in range(4):
                                b = n * 4 + bb
                                nc.tensor.transpose(
                                    tp[:, bb * P:(bb + 1) * P],
                                    xbs[b][:, hc * P:(hc + 1) * P],
                                    identh[:])
                            if hc % 2 == 0:
                                nc.vector.tensor_copy(xT[hc][:, sl], tp[:])
                            else:
                                nc.scalar.activation(xT[hc][:, sl], tp[:],
                                                     AF.Copy)
                    for b in range(TB):
                        ofs = psXr.tile([P, E], F32, tag="ofs")
                        nc.tensor.matmul(ofs[:], slb_sb[:, b * P:(b + 1) * P],
                                         totals[:], start=True, stop=True)
                        nc.vector.tensor_tensor(out=grank[b][:],
                                                in0=pre_sb[b][:], in1=ofs[:],
                                                op=ALU.add)
                        gm = px2.tile([P, E], F32, tag="gm")
                        nc.vector.tensor_scalar(out=gm[:], in0=grank[b][:],
                                                scalar1=float(CAP),
                                                scalar2=BIG,
                                                op0=ALU.is_gt, op1=ALU.mult)
                        nc.vector.tensor_add(grank[b][:], grank[b][:], gm[:])
                        um = px2.tile([P, E], F32, tag="um")
                        nc.vector.tensor_scalar(out=um[:], in0=selb[b][:],
                                                scalar1=-BIG, scalar2=BIG,
                                                op0=ALU.mult, op1=ALU.add)
                        nc.vector.tensor_add(grank[b][:], grank[b][:], um[:])
                # token lists (grank is 1-based; -1 folded into scatter base)
                sent = px.tile([P, 1], I32)
                nc.vector.memset(sent[:], 1000000)
                for kk in range(2 * CAP // P):
                    nc.sync.dma_start(tok_lists[kk * P:(kk + 1) * P, :],
                                      sent[:])
                for b in range(TB):
                    tok = px2.tile([P, 1], I32, tag="tok")
                    nc.gpsimd.iota(tok[:], pattern=[[0, 1]], base=b * P,
                                   channel_multiplier=1)
                    for ei in range(2):
                        gsel = px2.tile([P, E], F32, tag="gsel")
                        nc.vector.tensor_mul(gsel[:], grank[b][:], em[ei][:])
                        ridx = px2.tile([P, 1], F32, tag="ridx")
                        nc.vector.reduce_sum(ridx[:], gsel[:], axis=AX.X)
                        nc.vector.tensor_scalar_add(ridx[:], ridx[:],
                                                    float(ei * CAP - 1))
                        ridx_i = px2.tile([P, 1], I32, tag="ridxi")
                        nc.vector.tensor_copy(ridx_i[:], ridx[:])
                        nc.gpsimd.indirect_dma_start(
                            out=tok_lists[:],
                            out_offset=bass.IndirectOffsetOnAxis(
                                ap=ridx_i[:, :1], axis=0),
                            in_=tok[:], in_offset=None,
                            bounds_check=2 * CAP - 1, oob_is_err=False)

            with tc.tile_pool(name="pg", bufs=1) as pg, \
                 tc.tile_pool(name="pg2", bufs=2) as pg2:
                idx_sb = [[pg.tile([P, 1], I32, tag=f"idx{ei}_{k}",
                                   name=f"idx{ei}_{k}") for k in range(2)]
                          for ei in range(2)]
                gxT = [pg.tile([P, HC * 2 * P], F16, tag=f"gxT{ei}",
                               name=f"gxT{ei}") for ei in range(2)]
                wcol = [[pg.tile([P, 1], F32, tag=f"wcol{ei}_{k}",
                                 name=f"wcol{ei}_{k}") for k in range(2)]
                        for ei in range(2)]

                # ======== Shared expert ========
                hs = [pg.tile([P, T], F16, tag=f"hs{sp}", name=f"hs{sp}")
                      for sp in range(2)]
                with tc.tile_pool(name="psS", bufs=1, space="PSUM") as psS:
                    gps = [psS.tile([P, T], F32, tag=f"sgp{sp}",
                                    name=f"sgp{sp}") for sp in range(2)]
                    ups = [psS.tile([P, T], F32, tag=f"sup{sp}",
                                    name=f"sup{sp}") for sp in range(2)]
                    for hc in range(HC):
                        for sp in range(2):
                            c0 = hc * ISC + sp * P
                            for nn in range(2):
                                sl = slice(nn * 512, (nn + 1) * 512)
                                nc.tensor.matmul(gps[sp][:, sl],
                                                 wsg_sb[:, c0:c0 + P],
                                                 xT[hc][:, sl],
                                                 start=(hc == 0),
                                                 stop=(hc == HC - 1))
                                nc.tensor.matmul(ups[sp][:, sl],
                                                 wsu_sb[:, c0:c0 + P],
                                                 xT[hc][:, sl],
                                                 start=(hc == 0),
                                                 stop=(hc == HC - 1))
                    for sp in range(2):
                        ga = pg2.tile([P, T], F16, tag="ga")
                        nc.scalar.activation(ga[:], gps[sp][:], AF.Silu)
                        nc.vector.tensor_mul(hs[sp][:], ga[:], ups[sp][:])
                with tc.tile_pool(name="psS3", bufs=2, space="PSUM") as psS3:
                    for nn in range(2):
                        for tb_ in range(TB):
                            ps3 = psS3.tile([P, 1024], F32, tag="psSd")
                            for sp in range(2):
                                for q2 in range(2):
                                    s2 = slice(q2 * 512, (q2 + 1) * 512)
                                    nc.tensor.matmul(
                                        ps3[:, s2],
                                        hs[sp][:, tb_ * P:(tb_ + 1) * P],
                                        wsd_sb[sp][:, nn * 1024 + q2 * 512:
                                                    nn * 1024 + (q2 + 1) * 512],
                                        start=(sp == 0), stop=(sp == 1))
                            sd = pg2.tile([P, 1024], F16, tag="sd")
                            nc.vector.tensor_copy(sd[:], ps3[:])
                            nc.sync.dma_start(
                                rs2_in[tb_ * P:(tb_ + 1) * P,
                                       nn * 1024:(nn + 1) * 1024], sd[:])

                # ======== Expert gathers (gpsimd overlaps shared) ========
                with tc.tile_pool(name="psFt", bufs=2, space="PSUM") as psFt:
                    for ei in range(2):
                        for k in range(2):
                            nc.sync.dma_start(
                                idx_sb[ei][k][:],
                                tok_lists[ei * CAP + k * P:
                                          ei * CAP + (k + 1) * P, :])
                            gx = pg2.tile([P, PW], F16, tag="gx")
                            nc.vector.memset(gx[:], 0.0)
                            nc.gpsimd.indirect_dma_start(
                                out=gx[:], out_offset=None,
                                in_=x_tm[:],
                                in_offset=bass.IndirectOffsetOnAxis(
                                    ap=idx_sb[ei][k][:, :1], axis=0),
                                bounds_check=T - 1, oob_is_err=False)
                            wtmp = pg2.tile([P, E], F32, tag="wtmp")
                            nc.vector.tensor_mul(wtmp[:], gx[:, H:PW],
                                                 em[ei][:])
                            nc.vector.reduce_sum(wcol[ei][k][:], wtmp[:],
                                                 axis=AX.X)
                            for g4 in range(4):
                                tp = psFt.tile([P, 512], F16, tag="tpF")
                                for i in range(4):
                                    hc = g4 * 4 + i
                                    nc.tensor.transpose(
                                        tp[:, i * P:(i + 1) * P],
                                        gx[:, hc * P:(hc + 1) * P],
                                        identh[:])
                                dst = gxT[ei][:, k * HC * P + g4 * 512:
                                              k * HC * P + (g4 + 1) * 512]
                                if g4 % 2 == 0:
                                    nc.vector.tensor_copy(dst, tp[:])
                                else:
                                    nc.scalar.activation(dst, tp[:], AF.Copy)

                # ======== Experts ========
                for ei in range(2):
                    with tc.tile_pool(name=f"psF1{ei}", bufs=1,
                                      space="PSUM") as psF1:
                        h_tm = [pg.tile([P, I], F16, tag=f"htm{k}",
                                        name=f"htm{ei}_{k}")
                                for k in range(2)]
                        g_ps = [psF1.tile([P, I], F32, tag=f"fg{k}",
                                          name=f"fg{k}_{ei}")
                                for k in range(2)]
                        u_ps = [psF1.tile([P, I], F32, tag=f"fu{k}",
                                          name=f"fu{k}_{ei}")
                                for k in range(2)]
                        for hc in range(HC):
                            if ei == 0 and hc < 5:
                                wg, wu = wgl[hc], wul[hc]
                            else:
                                wg = pfw.tile([P, I], F16, tag="wg", bufs=5,
                                              name=f"wg{ei}_{hc}")
                                nc.sync.dma_start(
                                    wg[:],
                                    ex["we_g"][ei, hc * P:(hc + 1) * P, :])
                                wu = pfw.tile([P, I], F16, tag="wu", bufs=5,
                                              name=f"wu{ei}_{hc}")
                                nc.sync.dma_start(
                                    wu[:],
                                    ex["we_u"][ei, hc * P:(hc + 1) * P, :])
                            for k in range(2):
                                c0 = k * HC * P + hc * P
                                for nn in range(2):
                                    sl = slice(nn * 512, (nn + 1) * 512)
                                    nc.tensor.matmul(g_ps[k][:, sl],
                                                     gxT[ei][:, c0:c0 + P],
                                                     wg[:, sl],
                                                     start=(hc == 0),
                                                     stop=(hc == HC - 1))
                                    nc.tensor.matmul(u_ps[k][:, sl],
                                                     gxT[ei][:, c0:c0 + P],
                                                     wu[:, sl],
                                                     start=(hc == 0),
                                                     stop=(hc == HC - 1))
                        for k in range(2):
                            sg = pg2.tile([P, I], F16, tag="sg")
                            nc.scalar.activation(sg[:], g_ps[k][:], AF.Silu)
                            nc.vector.tensor_mul(h_tm[k][:], sg[:],
                                                 u_ps[k][:])
                    h_sb = pg.tile([P, IP * 2 * P], F16, tag=f"hsb{ei}",
                                   name=f"hsb{ei}")
                    with tc.tile_pool(name=f"psF2{ei}", bufs=2,
                                      space="PSUM") as psF2:
                        for k in range(2):
                            for g4 in range(2):
                                tp = psF2.tile([P, 512], F16, tag="tpF2")
                                for i in range(4):
                                    ip = g4 * 4 + i
                                    nc.tensor.transpose(
                                        tp[:, i * P:(i + 1) * P],
                                        h_tm[k][:, ip * P:(ip + 1) * P],
                                        identh[:])
                                dst = h_sb[:, k * IP * P + g4 * 512:
                                           k * IP * P + (g4 + 1) * 512]
                                if g4 % 2 == 0:
                                    nc.vector.tensor_copy(dst, tp[:])
                                else:
                                    nc.scalar.activation(dst, tp[:],
                                                         AF.Copy)
                    with tc.tile_pool(name=f"psF3{ei}", bufs=2,
                                      space="PSUM") as psF3:
                        for k in range(2):
                            out_sb = pg2.tile([P, H], F16, tag="outsb",
                                              name=f"outsb{ei}{k}")
                            for nn in range(2):
                                sl = slice(nn * 1024, (nn + 1) * 1024)
                                psd = psF3.tile([P, 1024], F32, tag="fd")
                                for ip in range(IP):
                                    c0 = k * IP * P + ip * P
                                    for q2 in range(2):
                                        s2 = slice(q2 * 512, (q2 + 1) * 512)
                                        nc.tensor.matmul(
                                            psd[:, s2], h_sb[:, c0:c0 + P],
                                            wd_res[ei * IP + ip][
                                                :, nn * 1024 + q2 * 512:
                                                nn * 1024 + (q2 + 1) * 512],
                                            start=(ip == 0),
                                            stop=(ip == IP - 1))
                                nc.vector.tensor_scalar_mul(
                                    out_sb[:, sl], psd[:],
                                    wcol[ei][k][:, :1])
                            nc.gpsimd.indirect_dma_start(
                                out=rs2_in[:],
                                out_offset=bass.IndirectOffsetOnAxis(
                                    ap=idx_sb[ei][k][:, :1], axis=0),
                                in_=out_sb[:], in_offset=None,
                                bounds_check=T - 1, oob_is_err=False,
                                compute_op=ALU.add)

            nc.gpsimd.collective_compute(
                "ReduceScatter", ALU.add, ins=[rs2_in.opt()],
                outs=[rs2_out.opt()], replica_groups=RG)
            with tc.tile_pool(name="pz", bufs=1) as pz:
                fin = pz.tile([P, H], F16)
                nc.sync.dma_start(fin[:], rs2_out[:])
                fin32 = pz.tile([P, H], F32)
                nc.vector.tensor_copy(fin32[:], fin[:])
                nc.sync.dma_start(out_slice[:], fin32[:])


_CACHE = {}


def _build():
    key = "nc"
    if key in _CACHE:
        return _CACHE[key]
    nc = bacc.Bacc("TRN2", target_bir_lowering=False, debug=False,
                   num_devices=NCN)
    with tile.TileContext(nc) as tc:
        _emit(nc, tc)
    nc.compile()
    _CACHE[key] = nc
    return nc


def _host_prep(inputs):
    f16 = np.float16
    pos = np.asarray(inputs["positions"]).astype(np.float64)
    hid = np.asarray(inputs["hidden_states"], np.float32)
    w_in = np.asarray(inputs["w_in_ln"], np.float32)
    w_post = np.asarray(inputs["w_post_ln"], np.float32)
    wq = np.asarray(inputs["wq"], np.float32) * w_in[:, None]
    wk = np.asarray(inputs["wk"], np.float32) * w_in[:, None]
    wv = np.asarray(inputs["wv"], np.float32) * w_in[:, None]
    wo = np.asarray(inputs["wo"], np.float32)
    gate_w = np.asarray(inputs["gate_w"], np.float32) * w_post[None, :]
    gate_b = np.asarray(inputs["gate_bias"], np.float32).reshape(1, E)
    we_g = (np.asarray(inputs["we_gate"], np.float32)
            * w_post[None, :, None]).astype(f16)
    we_u = (np.asarray(inputs["we_up"], np.float32)
            * w_post[None, :, None]).astype(f16)
    we_d = np.asarray(inputs["we_down"], np.float32).astype(f16)
    ws_g = np.asarray(inputs["ws_gate"], np.float32) * w_post[:, None]
    ws_u = np.asarray(inputs["ws_up"], np.float32) * w_post[:, None]
    ws_d = np.asarray(inputs["ws_down"], np.float32).astype(f16)

    inv_freq = 1.0 / (THETA ** (np.arange(0, D, 2, dtype=np.float64) / D))
    f = pos[None, :] * inv_freq[:, None]
    cos2, sin2 = np.cos(f), np.sin(f)
    cosT = np.repeat(cos2, 2, axis=0).astype(np.float32)
    sinT = np.empty((D, T), np.float32)
    sinT[0::2] = -sin2
    sinT[1::2] = sin2
    s = 1.0 / np.sqrt(D)
    cosq, sinq = (cosT * s).astype(f16), (sinT * s).astype(f16)
    cosk, sink = cosT.astype(f16), sinT.astype(f16)

    ii = np.arange(P)
    diagmask = np.where(ii[:, None] >= ii[None, :], 0.0, NEG).astype(f16)
    ident = np.eye(P, dtype=np.float32)
    ut_in = np.triu(np.ones((P, P), np.float32)).astype(f16)
    slb_in = np.zeros((8, TB * P), np.float32)
    for b in range(TB):
        slb_in[:b, b * P:(b + 1) * P] = 1.0
    slb_in = slb_in.astype(f16)
    perm = np.zeros((P, P), np.float32)
    for i in range(0, P, 2):
        perm[i, i + 1] = 1.0
        perm[i + 1, i] = 1.0

    # packed (chunk-major) stationary layouts: [128, HC*width]
    def pack_pk(w, width):  # w: [H, width]
        return np.ascontiguousarray(
            w.reshape(HC, P, width).transpose(1, 0, 2).reshape(P, HC * width))

    gate_w_pk = pack_pk(gate_w.T.astype(np.float32), E)

    maps = []
    for c in range(NCN):
        g = c // 2
        w_qkv = pack_pk(np.concatenate([
            wq[:, 2 * c * D:(2 * c + 1) * D],
            wq[:, (2 * c + 1) * D:(2 * c + 2) * D],
            wk[:, g * D:(g + 1) * D],
            wv[:, g * D:(g + 1) * D]], axis=1), 512).astype(f16)
        em0 = np.zeros((P, E), np.float32)
        em0[:, 2 * c] = 1.0
        em1 = np.zeros((P, E), np.float32)
        em1[:, 2 * c + 1] = 1.0
        maps.append({
            "hid": hid,
            "hid_slice": np.ascontiguousarray(hid[c * P:(c + 1) * P]),
            "w_qkv_pk": w_qkv,
            "wo0": np.ascontiguousarray(wo[2 * c * D:(2 * c + 1) * D]).astype(f16),
            "wo1": np.ascontiguousarray(
                wo[(2 * c + 1) * D:(2 * c + 2) * D]).astype(f16),
            "cosq": cosq, "sinq": sinq, "cosk": cosk, "sink": sink,
            "permh": perm.astype(f16), "identh_in": ident.astype(f16),
            "identr_in": ident, "diagmask": diagmask,
            "gate_w_pk": gate_w_pk,
            "gate_b": np.broadcast_to(gate_b, (P, E)).astype(np.float32).copy(),
            "emask0": em0, "emask1": em1,
            "ut_in": ut_in, "slb_in": slb_in,
            "ws_g_pk": pack_pk(
                ws_g[:, c * ISC:(c + 1) * ISC].astype(np.float32), ISC
            ).astype(f16),
            "ws_u_pk": pack_pk(
                ws_u[:, c * ISC:(c + 1) * ISC].astype(np.float32), ISC
            ).astype(f16),
            "ws_d": np.ascontiguousarray(ws_d[c * ISC:(c + 1) * ISC]),
            "we_g": np.ascontiguousarray(we_g[2 * c:2 * c + 2]),
            "we_u": np.ascontiguousarray(we_u[2 * c:2 * c + 2]),
            "we_d": np.ascontiguousarray(we_d[2 * c:2 * c + 2]),
        })
    return maps


def kernel(trace=False, **inputs):
    nc = _build()
    maps = _host_prep(inputs)
    res = bass_utils.run_bass_kernel_spmd(
        nc, maps, core_ids=list(range(NCN)), trace=trace)
    out = np.concatenate([res.results[c]["out_slice"] for c in range(NCN)], 0)
    resid = np.concatenate([res.results[c]["res_slice"] for c in range(NCN)], 0)
    kernel.last_results = res
    return out, resid
